# revision 1
# baseline (speedup 1.0000x reference)
"""Trainium2 Bass kernel for nn_Attention_24008867185039.

Reference computation (B=2, N=2048, DIM=1024, 16 heads x 64):
    q = x @ Wq ; k, v = split(x @ Wkv) ; per-head softmax(q k^T / sqrt(64)) v
    out = attn_out @ Wo + bo
(mask is all-ones per the problem spec, so masking is a no-op.)

Sharding (8 cores): data-parallel over batch (2) x tensor-parallel over 4
head-groups of 4 heads. Each core computes, for its (b, head-group):
  - Q^T, K^T projections [256, 2048] (d' on partitions -> ready for attention)
  - V projection [2048, 256] (natural layout, + a ones column per head so the
    PV matmul also produces softmax row-sums for free)
  - flash-style attention per head pair (exp on scalar engine, unnormalized
    accumulation, normalization folded in as a per-column reciprocal mul)
  - partial output projection against its 256-row slice of Wo
Host sums the 4 Wo partials per batch and adds the bias.

Precision: x and the Q/K/V projection weights load as bf16 (halves the input
DMA, which gates the projection phase); Q^T/K^T/attention-output/Wo stay
float32r (full PE rate at N>=256, ~TF32 accuracy); exp(S) and V use bf16 for
the PV matmul (elementwise errors average out over the 2048-long softmax
sums). End-to-end max relative error vs the fp32 reference: 3.5e-3.

Schedule (per core, cost-model 201us): the scalar engine's exp stream is the
pacing resource (128 ops x [128,1024] ~= 133us). Only a minimal projection
prefix (K^T(m0), Q^T(m0,chunk0), V) runs before attention starts; the other
11 projection chains and every chunk's (one-chunk-deferred) output projection
drip through the attention blocks' PE slack on a dedicated PSUM bank, so the
PE stream never idles waiting on work that the in-order engines could have
reordered around.
"""

import sys

sys.path.insert(0, "/opt/trn_rl_repo")

import numpy as np

B, N, DIM, HEADS, DH = 2, 2048, 1024, 16, 64
HPG = 4                 # heads per core (head group)
DGRP = HPG * DH         # 256: per-core slice of the inner dim
NCORES = 8
KT = DIM // 128         # 8 contraction tiles for projections
NT = N // 128           # 16 sequence tiles of 128
NI = N // 512           # 4 query chunks of 512
MT = DGRP // 128        # 2 head-pair tiles per core

_CACHE = {}


def build_program(repeats=1):
    import concourse.mybir as mybir
    import concourse.tile as tile
    from concourse import bacc

    f32 = mybir.dt.float32

    nc = bacc.Bacc("TRN2", target_bir_lowering=False, debug=False,
                   num_devices=NCORES)

    bf16 = mybir.dt.bfloat16
    xt_d = nc.dram_tensor("xt", [DIM, N], bf16, kind="ExternalInput").ap()
    wq_d = nc.dram_tensor("wq", [DIM, DGRP], bf16, kind="ExternalInput").ap()
    wk_d = nc.dram_tensor("wk", [DIM, DGRP], bf16, kind="ExternalInput").ap()
    wv_d = nc.dram_tensor("wv", [DIM, DGRP], bf16, kind="ExternalInput").ap()
    wo_d = nc.dram_tensor("wo", [DGRP, DIM], f32, kind="ExternalInput").ap()
    part_d = nc.dram_tensor("part", [N, DIM], f32, kind="ExternalOutput").ap()

    with tile.TileContext(nc) as tc:
        for rep in range(repeats):
            _emit_body(nc, tc, xt_d, wq_d, wk_d, wv_d, wo_d, part_d,
                       tag=f"r{rep}")

    nc.compile()
    return nc


def _emit_body(nc, tc, xt_d, wq_d, wk_d, wv_d, wo_d, part_d, tag):
    import concourse.mybir as mybir

    f32 = mybir.dt.float32
    bf16 = mybir.dt.bfloat16
    f32r = mybir.dt.float32r
    Exp = mybir.ActivationFunctionType.Exp

    xt_t = xt_d.rearrange("(t p) n -> t p n", p=128)
    wq_t = wq_d.rearrange("(t p) d -> p t d", p=128)
    wk_t = wk_d.rearrange("(t p) d -> p t d", p=128)
    wv_t = wv_d.rearrange("(t p) d -> p t d", p=128)
    wo_t = wo_d.rearrange("(t p) d -> t p d", p=128)

    def r(ap):
        return ap.bitcast(f32r)

    with nc.allow_low_precision(reason="float32r rounding is intentional"):
        with tc.tile_pool(name=f"persist{tag}", bufs=1) as pp, \
             tc.tile_pool(name=f"small{tag}", bufs=4) as sp, \
             tc.tile_pool(name=f"epool{tag}", bufs=6) as ep:

            # Persistent SBUF tensors
            qt = [[pp.tile([128, 512], f32r, name=f"qt{m}_{i}")
                   for i in range(NI)] for m in range(MT)]
            kt = [pp.tile([128, N], f32r, name=f"kt{m}") for m in range(MT)]
            # V with a ones column per head: [128, 4 heads x (64 d + 1)]
            vsb = [pp.tile([128, HPG * (DH + 1)], bf16, name=f"vsb{j}")
                   for j in range(NT)]
            wo_sb = [pp.tile([128, DIM], f32r, name=f"wo_sb{m}")
                     for m in range(MT)]

            # ---- Phase A: x/weight load, m0 projections, V ----------------
            with tc.tile_pool(name=f"xw{tag}", bufs=1) as xp:
                xt_sb = [xp.tile([128, N], bf16, name=f"xt{k}")
                         for k in range(KT)]
                # weight tiles hold all k-tiles side by side: [128, k, d]
                wq_sb = xp.tile([128, KT, DGRP], bf16, name="wq_sb")
                wk_sb = xp.tile([128, KT, DGRP], bf16, name="wk_sb")
                wv_sb = xp.tile([128, KT, DGRP], bf16, name="wv_sb")
                # one DMA per weight matrix; x k-tiles interleaved so the
                # projection accumulators start as soon as data lands
                nc.sync.dma_start(out=wk_sb[:, 0:2, :], in_=wk_t[:, 0:2, :])
                nc.sync.dma_start(out=xt_sb[0][:, 0:512],
                                  in_=xt_t[0][:, 0:512])
                nc.sync.dma_start(out=wk_sb[:, 2:KT, :],
                                  in_=wk_t[:, 2:KT, :])
                nc.sync.dma_start(out=xt_sb[0][:, 512:N],
                                  in_=xt_t[0][:, 512:N])
                nc.sync.dma_start(out=wq_sb[:], in_=wq_t)
                nc.sync.dma_start(out=xt_sb[1][:], in_=xt_t[1])
                nc.sync.dma_start(out=wv_sb[:], in_=wv_t)
                for k in range(2, KT):
                    nc.sync.dma_start(out=xt_sb[k][:], in_=xt_t[k])
                # wo is not read until the first output projection (~70us
                # in), so it loads after everything the prefix depends on
                for m in range(MT):
                    nc.sync.dma_start(out=wo_sb[m][:], in_=r(wo_t[m]))

                with tc.tile_pool(name=f"psA{tag}", bufs=2, space="PSUM") as pa:
                    # minimal prefix before attention can start: K^T(m0)
                    # in full, Q^T(m0) for the first query chunk, and V.
                    # Everything else drips through attention's PE slack.
                    for ich in range(NI):
                        isl = slice(ich * 512, (ich + 1) * 512)
                        k_ps = pa.tile([128, 512], f32, name="k_ps", bufs=3)
                        for k in range(KT):
                            nc.tensor.matmul(
                                k_ps[:], wk_sb[:, k, 0:128],
                                xt_sb[k][:, isl],
                                start=(k == 0), stop=(k == KT - 1))
                        nc.vector.tensor_copy(out=kt[0][:, isl], in_=k_ps[:])
                    q_ps = pa.tile([128, 512], f32, name="q_ps", bufs=2)
                    for k in range(KT):
                        nc.tensor.matmul(
                            q_ps[:], wq_sb[:, k, 0:128],
                            xt_sb[k][:, 0:512],
                            start=(k == 0), stop=(k == KT - 1))
                    nc.scalar.copy(out=qt[0][0][:], in_=q_ps[:])
                    # two of stream-B's projection chains run here, filling
                    # DMA-gated PE gaps during the x load
                    for (wsb, msl, isl, dst) in (
                            (wq_sb, slice(0, 128), slice(512, 1024),
                             qt[0][1][:]),
                            (wk_sb, slice(128, 256), slice(0, 512),
                             kt[1][:, 0:512])):
                        a_ps = pa.tile([128, 512], f32, name="a_ps",
                                       tag="achain", bufs=1)
                        for k in range(KT):
                            nc.tensor.matmul(
                                a_ps[:], wsb[:, k, msl],
                                xt_sb[k][:, isl],
                                start=(k == 0), stop=(k == KT - 1))
                        nc.vector.tensor_copy(out=dst, in_=a_ps[:])
                    # V: [n 128, d' 256] tiles (natural layout)
                    for j in range(NT):
                        jsl = slice(j * 128, (j + 1) * 128)
                        v_ps = pa.tile([128, DGRP], f32, name="v_ps",
                                       bufs=2)
                        for k in range(KT):
                            nc.tensor.matmul(
                                v_ps[:], xt_sb[k][:, jsl], wv_sb[:, k, :],
                                start=(k == 0), stop=(k == KT - 1))
                        v3 = vsb[j].rearrange("p (h c) -> p h c", h=HPG)
                        nc.vector.tensor_copy(
                            out=v3[:, :, 0:DH],
                            in_=v_ps.rearrange("p (h c) -> p h c", h=HPG))
                        nc.vector.memset(v3[:, :, DH:DH + 1], 1.0)

                # ---- Phase B+C: attention per (query chunk, head pair),
                # with each chunk's output projection deferred one chunk
                # and interleaved into the next chunk's exp-paced loop. ----
                with tc.tile_pool(name=f"psB{tag}", bufs=2, space="PSUM") as pb, \
                     tc.tile_pool(name=f"osb{tag}", bufs=4) as op:

                    def make_outproj(ig, ot_pair, last=False):
                        # Deferred output projection for (stream m, chunk
                        # ig): 8 emit-thunks + ship(s), interleaved into
                        # later attention so they fill PE slack (attention
                        # is exp-paced on the scalar engine). Results
                        # collect into one wide SBUF buffer per chunk.
                        out_sb = op.tile([128, 4096], f32, name="out_sb",
                                         tag="out_sb", bufs=2)

                        def group(gi):
                            jt, dch = gi // 2, gi % 2
                            lsl = slice(jt * 128, (jt + 1) * 128)
                            dsl = slice(dch * 512, (dch + 1) * 512)
                            # final chunk: attention banks are dead, so
                            # alternate onto the o_ps tag to double-buffer
                            if last and gi % 2 == 1:
                                out_ps = pb.tile([128, 512], f32,
                                                 name="out_psb", tag="o_ps",
                                                 bufs=2)
                            else:
                                out_ps = pb.tile([128, 512], f32,
                                                 name="out_ps", tag="out_ps",
                                                 bufs=1)
                            for m in range(MT):
                                nc.tensor.matmul(
                                    out_ps[:, 0:512], ot_pair[m][:, lsl],
                                    wo_sb[m][:, dsl],
                                    start=(m == 0), stop=(m == MT - 1))
                            csl = slice(jt * 1024 + dch * 512,
                                        jt * 1024 + (dch + 1) * 512)
                            # mid-stream: keep the exp-pacing scalar engine
                            # copy-free; final chunk: it is idle, so split
                            if last and dch == 0:
                                nc.scalar.copy(out=out_sb[:, csl],
                                               in_=out_ps[:, 0:512])
                            else:
                                nc.vector.tensor_copy(out=out_sb[:, csl],
                                                      in_=out_ps[:, 0:512])

                        def ship(jt0, jt1):
                            part_v = part_d[ig * 512 + jt0 * 128:
                                            ig * 512 + jt1 * 128,
                                            :].rearrange(
                                "(jt p) d -> p jt d", p=128)
                            nc.sync.dma_start(
                                out=part_v,
                                in_=out_sb[:, jt0 * 1024:jt1 * 1024]
                                .rearrange("p (jt d) -> p jt d",
                                           jt=jt1 - jt0))

                        thunks = []
                        for gi in range(8):
                            thunks.append(lambda gi=gi: group(gi))
                            if last and gi % 2 == 1:
                                jt = gi // 2
                                thunks.append(lambda jt=jt: ship(jt, jt + 1))
                        if not last:
                            thunks.append(lambda: ship(0, 4))
                        return thunks

                    pending = []

                    def attention(m, ig, oa_tag="o_ps"):
                        nonlocal pending
                        hA, hB = 2 * m, 2 * m + 1
                        vA = slice(hA * (DH + 1), hA * (DH + 1) + DH + 1)
                        vB = slice(hB * (DH + 1), hB * (DH + 1) + DH + 1)
                        ot_ig = op.tile([128, 512], f32r, name=f"ot{m}",
                                        tag="ot", bufs=6)
                        o_psA = pb.tile([DH + 1, 512], f32, name="o_psA",
                                        tag=oa_tag,
                                        bufs=(1 if oa_tag == "proj" else 2))
                        o_psB = pb.tile([DH + 1, 512], f32, name="o_psB",
                                        tag="o_ps", bufs=2)
                        for j in range(NT):
                            jsl = slice(j * 128, (j + 1) * 128)
                            s_ps = pb.tile([128, 1024], f32, name="s_ps",
                                           tag="s_ps")
                            # two heads run concurrently in disjoint PE
                            # row-groups (K=64 each)
                            nc.tensor.matmul(
                                s_ps[:, 0:512], kt[m][0:64, jsl],
                                qt[m][ig][0:64, :], start=True, stop=True)
                            nc.tensor.matmul(
                                s_ps[:, 512:1024], kt[m][64:128, jsl],
                                qt[m][ig][64:128, :], start=True, stop=True)
                            e_sb = ep.tile([128, 1024], bf16, name="e_sb")
                            nc.scalar.activation(out=e_sb[:], in_=s_ps[:],
                                                 func=Exp)
                            nc.tensor.matmul(
                                o_psA[:], vsb[j][:, vA], e_sb[:, 0:512],
                                start=(j == 0), stop=(j == NT - 1))
                            nc.tensor.matmul(
                                o_psB[:], vsb[j][:, vB], e_sb[:, 512:1024],
                                start=(j == 0), stop=(j == NT - 1))
                            if pending and j % 2 == 1:
                                pending.pop(0)()
                        # normalize: O^T[d, i] * (1 / rowsum[i])
                        for side, o_ps in ((0, o_psA), (1, o_psB)):
                            rr = sp.tile([1, 512], f32, name="rr", tag="rr")
                            nc.vector.reciprocal(rr[0:1, :],
                                                 o_ps[DH:DH + 1, :])
                            rb_sb = sp.tile([DH, 512], f32, name="rb_sb",
                                            tag="rb_sb")
                            nc.gpsimd.partition_broadcast(rb_sb[:],
                                                          rr[0:1, :])
                            nc.vector.tensor_mul(
                                out=ot_ig[side * DH:(side + 1) * DH, :],
                                in0=o_ps[0:DH, :], in1=rb_sb[:])
                        return ot_ig

                    def proj_chain(w_sb, msl, dst, use_act, isl):
                        # one [128,512] projection accumulation on the
                        # dedicated proj bank, dripped between attention
                        # blocks
                        p_ps = pb.tile([128, 512], f32, name="p_ps",
                                       tag="proj", bufs=1)
                        for k in range(KT):
                            nc.tensor.matmul(
                                p_ps[:], w_sb[:, k, msl], xt_sb[k][:, isl],
                                start=(k == 0), stop=(k == KT - 1))
                        if use_act:
                            nc.scalar.copy(out=dst, in_=p_ps[:])
                        else:
                            nc.vector.tensor_copy(out=dst, in_=p_ps[:])

                    def q_chain(m, ich):
                        msl = slice(m * 128, (m + 1) * 128)
                        isl = slice(ich * 512, (ich + 1) * 512)
                        proj_chain(wq_sb, msl, qt[m][ich][:], False, isl)

                    def k_chain(ich):
                        isl = slice(ich * 512, (ich + 1) * 512)
                        proj_chain(wk_sb, slice(128, 256),
                                   kt[1][:, isl], False, isl)

                    # remaining projections, in dependency order: Q(m0)
                    # chunks feed the next m0 attention; K(m1)/Q(m1) must
                    # all land before stream m1 starts
                    chains = [lambda: q_chain(0, 2), lambda: k_chain(1),
                              lambda: q_chain(0, 3), lambda: k_chain(2),
                              lambda: k_chain(3), lambda: q_chain(1, 0)]
                    chains += [lambda i=i: q_chain(1, i) for i in (1, 2, 3)]

                    ot0 = []
                    for ig in range(NI):
                        ot0.append(attention(0, ig))
                        for _ in range(2):
                            if chains:
                                chains.pop(0)()
                    # leftover Q(m1) chains drip inside attention(1,0),
                    # which carries no output projection yet
                    pending = chains
                    for ig in range(NI):
                        # after the projection chains retire (during ig 0),
                        # the proj bank serves as a third O-accumulator slot
                        # so new chunks never wait on the previous chunk's
                        # normalization reads
                        otB = attention(1, ig,
                                        oa_tag=("proj" if ig >= 1
                                                else "o_ps"))
                        for fn in pending:
                            fn()
                        pending = make_outproj(ig, (ot0[ig], otB),
                                               last=(ig == NI - 1))
                    for fn in pending:
                        fn()


def _get_nc():
    if "nc" not in _CACHE:
        _CACHE["nc"] = build_program()
    return _CACHE["nc"]


def make_in_maps(x, Wq, Wkv, Wo):
    import ml_dtypes

    bf16 = ml_dtypes.bfloat16
    scale = DH ** -0.5
    x = np.asarray(x, dtype=np.float32)
    Wq = np.asarray(Wq, dtype=np.float32)
    Wkv = np.asarray(Wkv, dtype=np.float32)
    Wo = np.asarray(Wo, dtype=np.float32)
    xt = [np.ascontiguousarray(x[b].T.astype(bf16)) for b in range(B)]
    in_maps = []
    for c in range(NCORES):
        b, hg = c // HPG, c % HPG
        sl = slice(hg * DGRP, (hg + 1) * DGRP)
        in_maps.append({
            "xt": xt[b],
            "wq": np.ascontiguousarray((Wq[:, sl] * scale).astype(bf16)),
            "wk": np.ascontiguousarray(
                Wkv[:, hg * DGRP:(hg + 1) * DGRP].astype(bf16)),
            "wv": np.ascontiguousarray(
                Wkv[:, DIM + hg * DGRP:DIM + (hg + 1) * DGRP].astype(bf16)),
            "wo": np.ascontiguousarray(Wo[sl, :]),
        })
    return in_maps


def combine_outputs(results, bo):
    out = np.zeros((B, N, DIM), dtype=np.float32)
    for c in range(NCORES):
        out[c // HPG] += results[c]["part"]
    out += np.asarray(bo, dtype=np.float32)
    return out


def kernel(x, mask, Wq, Wkv, Wo, bo):
    from concourse.bass_utils import run_bass_kernel_spmd

    nc = _get_nc()
    in_maps = make_in_maps(x, Wq, Wkv, Wo)
    res = run_bass_kernel_spmd(nc, in_maps, list(range(NCORES)))
    return combine_outputs(res.results, bo)



# revision 68
# speedup vs baseline: 1.1924x; 1.1924x over previous
"""Trainium2 Bass kernel for nn_Attention_24008867185039.

Reference computation (B=2, N=2048, DIM=1024, 16 heads x 64):
    q = x @ Wq ; k, v = split(x @ Wkv) ; per-head softmax(q k^T / sqrt(64)) v
    out = attn_out @ Wo + bo
(mask is all-ones per the problem spec, so masking is a no-op.)

Sharding (8 cores): data-parallel over batch (2) x tensor-parallel over 4
head-groups of 4 heads. Host sums the 4 Wo partials per batch, adds bias.

Cost-model shape (per core): the scalar engine's exp stream (128 ops of
[128,1024], ~137us) and the PE stream (~142us) are co-critical.  PE work is
minimized by exploiting that matmul cost is charged per MOVING row only
(stationary loads are free):
  - QK^T: stationary K-tile [64d, 128keys], moving Q [64d, 512q] -> S^T
    [128 keys, 512 q] per (head, j).  (d=64 makes 50% PE util unavoidable.)
  - PV: stationary exp-tile [128 keys, 128 q], moving V [128 keys, 65]
    (64 dims + a ones column that yields the softmax row-sum for free) ->
    O [128 q, 65] accumulated over the 16 key tiles.  65 moving rows per
    matmul instead of 512 halves the attention-PV cost vs the naive layout.
  - O lands as [q, d']: normalization is a native per-partition
    reciprocal + tensor_scalar multiply, then a PE transpose (128 rows)
    rebuilds O^T [d', q] for the (full-util) output projection.
Projections and the output projection are at the full-utilization floor.

Precision: x/Wq/Wkv load as bf16; S^T in f32 PSUM; exp/V/O^T/Wo in fp16
(same PE rate as bf16, 8x finer mantissa).

Schedule: dummy ident matmuls ramp the PE p-state while the first x/weight
DMAs land; a minimal prefix (K^T cols 0:128 of pair0 + Q^T(pair0,chunk0))
starts the exp stream ~7us in.  Every other projection chain, V tile,
transpose and output-projection group is a thunk dripped through the
attention windows' PE slack under a credit scheduler (~440ns of drip
budget per exp), with an explicit need() guard enforcing producer-before-
consumer emission order.  PV runs 3 key-tiles behind QK so the previous
window's normalization reads finish before the O accumulators are reused.
PSUM: s 2x[128,1024] + O-accum 2x[128,4x65] + a shared 2-bank ring for
proj chains / transposes / outproj tiles = 8 banks.
"""

import sys

sys.path.insert(0, "/opt/trn_rl_repo")

import numpy as np

B, N, DIM, HEADS, DH = 2, 2048, 1024, 16, 64
HPG = 4                 # heads per core (head group)
DGRP = HPG * DH         # 256: per-core slice of the inner dim
NCORES = 8
KT = DIM // 128         # 8 contraction tiles for projections
NT = N // 128           # 16 sequence tiles of 128
NI = N // 512           # 4 query chunks of 512
MT = DGRP // 128        # 2 head-pair tiles per core
LAG = 5                 # PV trails QK by this many key tiles
WARMUP = 36             # PE p-state ramp matmuls during the first DMAs
SLACK_NS = 440          # drip budget granted per exp op

_CACHE = {}


def build_program(repeats=1):
    import concourse.mybir as mybir
    import concourse.tile as tile
    from concourse import bacc

    f32 = mybir.dt.float32
    bf16 = mybir.dt.bfloat16
    fp16 = mybir.dt.float16

    nc = bacc.Bacc("TRN2", target_bir_lowering=False, debug=False,
                   num_devices=NCORES)

    xt_d = nc.dram_tensor("xt", [DIM, N], bf16, kind="ExternalInput").ap()
    wq_d = nc.dram_tensor("wq", [DIM, DGRP], bf16, kind="ExternalInput").ap()
    wk_d = nc.dram_tensor("wk", [DIM, DGRP], bf16, kind="ExternalInput").ap()
    wv_d = nc.dram_tensor("wv", [DIM, DGRP], bf16, kind="ExternalInput").ap()
    wo_d = nc.dram_tensor("wo", [DGRP, DIM], fp16, kind="ExternalInput").ap()
    part_d = nc.dram_tensor("part", [N, DIM], fp16,
                            kind="ExternalOutput").ap()

    with tile.TileContext(nc) as tc:
        for rep in range(repeats):
            _emit_body(nc, tc, xt_d, wq_d, wk_d, wv_d, wo_d, part_d,
                       tag=f"r{rep}")

    nc.compile()
    return nc


def _emit_body(nc, tc, xt_d, wq_d, wk_d, wv_d, wo_d, part_d, tag):
    import concourse.mybir as mybir
    from concourse.masks import make_identity
    from concourse.tile_rust import add_dep_helper

    f32 = mybir.dt.float32
    bf16 = mybir.dt.bfloat16
    fp16 = mybir.dt.float16
    Exp = mybir.ActivationFunctionType.Exp

    xt_t = xt_d.rearrange("(t p) n -> p t n", p=128)    # [128, KT, N]
    wq_t = wq_d.rearrange("(t p) d -> p t d", p=128)    # [128, KT, DGRP]
    wk_t = wk_d.rearrange("(t p) d -> p t d", p=128)
    wv_t = wv_d.rearrange("(t p) d -> p t d", p=128)
    wo_t = wo_d.rearrange("(t p) d -> t p d", p=128)    # [MT, 128, DIM]

    with nc.allow_low_precision(reason="fp16/bf16 rounding is intentional"):
        with tc.tile_pool(name=f"pp{tag}", bufs=1) as pp, \
             tc.tile_pool(name=f"sp{tag}", bufs=4) as sp, \
             tc.tile_pool(name=f"ep{tag}", bufs=11) as ep, \
             tc.tile_pool(name=f"osb{tag}", bufs=2) as op:

            # ---- persistent SBUF ------------------------------------------
            # x^T lives as one tile per 512-token chunk so consumers only
            # depend on the DMA that actually feeds them
            xt_sb = [pp.tile([128, KT, 512], bf16, name=f"xt_sb{c}")
                     for c in range(NI)]
            wq_sb = pp.tile([128, KT, DGRP], bf16, name="wq_sb")
            wk_sb = pp.tile([128, KT, DGRP], bf16, name="wk_sb")
            wv_sb = pp.tile([128, KT, DGRP], bf16, name="wv_sb")
            wo_sb = [pp.tile([128, DIM], fp16, name=f"wo_sb{m}")
                     for m in range(MT)]
            qt = [[pp.tile([128, 512], fp16, name=f"qt{m}_{i}")
                   for i in range(NI)] for m in range(MT)]
            kt = [pp.tile([128, N], fp16, name=f"kt{m}") for m in range(MT)]
            # V with a ones column per head: [128, 4 heads x (64 d + 1)]
            vsb = [pp.tile([128, HPG * (DH + 1)], fp16, name=f"vsb{j}")
                   for j in range(NT)]
            ident = pp.tile([128, 128], fp16, name="ident")
            # warmup operand, independent of ident so the PE ramp matmuls
            # don't wait on the Pool engine's startup memset queue
            wrm = pp.tile([128, 128], fp16, name="wrm")
            nc.vector.memset(wrm[:], 0.125)
            # touch Exp once so the activation table is resident before the
            # first real exp (the lazy load would otherwise delay it); uses
            # its own tiny tile so the warmup matmuls don't wait on it
            pre = pp.tile([1, 2], fp16, name="pre")
            nc.vector.memset(pre[:], 0.1)
            nc.scalar.activation(out=pre[0:1, 0:1], in_=pre[0:1, 1:2],
                                 func=mybir.ActivationFunctionType.Exp)
            make_identity(nc, ident[:])

            # ---- DMAs (ordered by first need) -----------------------------
            # All transfers serialize on the DMA-engine pool, and runs
            # under 512B get half bandwidth, so weights go as full-width
            # transfers and x's first chunk in two 256-col (512B-run) pieces
            nc.sync.dma_start(out=wk_sb[:], in_=wk_t)
            nc.sync.dma_start(out=xt_sb[0][:, :, 0:256],
                              in_=xt_t[:, :, 0:256])
            nc.sync.dma_start(out=wq_sb[:], in_=wq_t)
            nc.sync.dma_start(out=xt_sb[0][:, :, 256:512],
                              in_=xt_t[:, :, 256:512])
            nc.sync.dma_start(out=wv_sb[:], in_=wv_t)
            nc.sync.dma_start(out=xt_sb[1][:], in_=xt_t[:, :, 512:1024])
            nc.sync.dma_start(out=xt_sb[2][:], in_=xt_t[:, :, 1024:1536])
            nc.sync.dma_start(out=xt_sb[3][:], in_=xt_t[:, :, 1536:2048])
            for m in range(MT):
                nc.sync.dma_start(out=wo_sb[m][:], in_=wo_t[m])

            # ---- thunk scheduler (deadline-ordered drip queue) ------------
            import heapq

            def v3(j):
                return vsb[j].rearrange("p (h c) -> p h c", h=HPG)

            emitted = set()
            heap = []
            state = {"credit": 0.0, "seq": 0}

            def add_thunk(name, fn, cost, deadline):
                state["seq"] += 1
                heapq.heappush(heap, (deadline, state["seq"], name, fn,
                                      cost))

            def pop_one():
                _, _, name, fn, cost = heapq.heappop(heap)
                fn()
                emitted.add(name)
                # debt floor: a forced overdraw stalls the exp stream once;
                # later windows shouldn't keep paying for it
                state["credit"] = max(state["credit"] - cost, -1200.0)

            def drip(budget_pops=3, force=False):
                # cap stops surplus from bursting several projection chains
                # into one slot (which would starve the exp stream)
                state["credit"] = min(state["credit"] + SLACK_NS, 1800.0)
                n = 0
                while heap and n < budget_pops and (
                        force or state["credit"] >= heap[0][4]):
                    pop_one()
                    n += 1

            def need(name):
                while name not in emitted:
                    assert heap, f"thunk {name} was never queued"
                    pop_one()

            # ---- Phase A: warmup + minimal prefix -------------------------
            with tc.tile_pool(name=f"pa{tag}", bufs=1, space="PSUM") as pa:
                scratch = pa.tile([128, 128], f32, name="scratch")
                for _ in range(WARMUP):
                    nc.tensor.matmul(scratch[:], wrm[:], wrm[:],
                                     start=True, stop=True)
                # K^T(pair0) cols 0:128 only — just enough for QK(j=0)
                kc_ps = pa.tile([128, 128], f32, name="kc_ps")
                for k in range(KT):
                    nc.tensor.matmul(kc_ps[:], wk_sb[:, k, 0:128],
                                     xt_sb[0][:, k, 0:128],
                                     start=(k == 0), stop=(k == KT - 1))
                nc.vector.tensor_copy(out=kt[0][:, 0:128], in_=kc_ps[:])
                emitted.add("kt0p0")
                for _ in range(20):
                    nc.tensor.matmul(scratch[:], wrm[:], wrm[:],
                                     start=True, stop=True)
                # Q^T(pair0) chunk0, in two half-chains pipelined against
                # the two x column-piece DMAs
                for h, csl in enumerate((slice(0, 256), slice(256, 512))):
                    q_ps = pa.tile([128, 256], f32, name=f"q_ps{h}")
                    for k in range(KT):
                        nc.tensor.matmul(q_ps[:], wq_sb[:, k, 0:128],
                                         xt_sb[0][:, k, csl],
                                         start=(k == 0), stop=(k == KT - 1))
                    nc.vector.tensor_copy(out=qt[0][0][:, csl], in_=q_ps[:])
                    if h == 0:
                        for _ in range(19):
                            nc.tensor.matmul(scratch[:], wrm[:], wrm[:],
                                             start=True, stop=True)
                emitted.add("qt00")

            # ---- dripped projection thunks --------------------------------
            with tc.tile_pool(name=f"pb{tag}", bufs=2, space="PSUM") as pb:

                def proj_cols(w_sb, msl, ich, csl, dst_copy):
                    p_ps = pb.tile([128, 512], f32, name="p_ps", tag="op",
                                   bufs=2)
                    ncols = csl.stop - csl.start
                    for k in range(KT):
                        nc.tensor.matmul(p_ps[:, 0:ncols],
                                         w_sb[:, k, msl],
                                         xt_sb[ich][:, k, csl],
                                         start=(k == 0), stop=(k == KT - 1))
                    dst_copy(p_ps[:, 0:ncols])

                def kt_cols(m, ich, c0, c1):
                    gsl = slice(ich * 512 + c0, ich * 512 + c1)
                    proj_cols(
                        wk_sb, slice(m * 128, (m + 1) * 128), ich,
                        slice(c0, c1),
                        lambda p: nc.vector.tensor_copy(out=kt[m][:, gsl],
                                                        in_=p))

                def qt_cols(m, ich, c0, c1):
                    proj_cols(
                        wq_sb, slice(m * 128, (m + 1) * 128), ich,
                        slice(c0, c1),
                        lambda p: nc.vector.tensor_copy(
                            out=qt[m][ich][:, c0:c1], in_=p))

                def v_tile(pair, j):
                    csl = slice((j % 4) * 128, (j % 4) * 128 + 128)
                    msl = slice(pair * 128, (pair + 1) * 128)
                    v_ps = pb.tile([128, 512], f32, name="vp_ps", tag="op",
                                   bufs=2)
                    for k in range(KT):
                        nc.tensor.matmul(v_ps[:, 0:128],
                                         xt_sb[j // 4][:, k, csl],
                                         wv_sb[:, k, msl],
                                         start=(k == 0), stop=(k == KT - 1))
                    nc.vector.tensor_copy(
                        out=v3(j)[:, 2 * pair:2 * pair + 2, 0:DH],
                        in_=v_ps[:, 0:128].rearrange("p (h c) -> p h c", h=2))
                    nc.gpsimd.memset(
                        v3(j)[:, 2 * pair:2 * pair + 2, DH:DH + 1], 1.0)

                QRT, VCOST = 427.0, 427.0

                def add_qt(m, ich, name, dls):
                    # quarter chains; the full-chunk name goes on the last
                    # quarter so need(name) forces all four
                    for q in range(4):
                        add_thunk(name + ("" if q == 3 else f"_{q}"),
                                  lambda q=q: qt_cols(m, ich, q * 128,
                                                      (q + 1) * 128),
                                  QRT, dls[q])

                # prefix extension: kt0 key-pieces 1-3 run in the PE gaps
                # while the first DMAs land (emitted by the code below, not
                # dripped)
                def kt0_piece(j):
                    kt_cols(0, j // 4, (j % 4) * 128, (j % 4) * 128 + 128)

                for j in (1, 2, 3):
                    kt0_piece(j)
                    emitted.add(f"kt0p{j}")
                # pair-0 K^T in 128-col pieces so the forced drip in the
                # first window stays fine-grained
                for j in range(4, NT):
                    add_thunk(f"kt0p{j}", lambda j=j: kt0_piece(j),
                              VCOST, (0, j - 0.5))
                # V tiles whose PV consumer is deferred into the next window
                # spill their deadline there too, interleaved with the
                # deferred PV thunks that consume them
                for j in range(NT):
                    # spilled V tiles must sort BEFORE their deferred-PV
                    # consumer, which must sort before that window's norm
                    dl = (0, j + 4.7) if j <= 10 else (1, 0.4 * (j - 11))
                    add_thunk(f"v0_{j}", lambda j=j: v_tile(0, j), VCOST,
                              dl)
                add_qt(0, 1, "qt01", [(0, 8), (0, 9.7), (0, 11.4),
                                      (0, 13.1)])
                for h in range(2 * NI):
                    w_, o_ = (1, 6) if h < 2 else ((2, 6) if h < 4
                                                   else (3, 1))
                    add_thunk(f"kt1c{h // 2}" + ("" if h % 2 else "a"),
                              lambda h=h: kt_cols(1, h // 2, (h % 2) * 256,
                                                  (h % 2) * 256 + 256),
                              853.0, (w_, o_ + 3 * (h % 2 if h < 4
                                                    else h - 4)))
                add_qt(0, 2, "qt02", [(1, 7), (1, 9), (1, 11), (1, 13)])
                for j in range(0, 8):
                    add_thunk(f"v1_{j}", lambda j=j: v_tile(1, j), VCOST,
                              (2, 4 + 0.9 * j))
                add_qt(0, 3, "qt03", [(2, 7), (2, 9), (2, 11), (2, 13)])
                for j in range(8, NT):
                    dl = ((3, 2 + 1.2 * (j - 8)) if j <= 10
                          else (5, 0.4 * (j - 11)))
                    add_thunk(f"v1_{j}", lambda j=j: v_tile(1, j), VCOST,
                              dl)
                add_qt(1, 0, "qt10", [(3, 5), (3, 7.5), (3, 10), (3, 12.5)])
                add_qt(1, 1, "qt11", [(4, 5), (4, 7), (4, 9), (4, 11)])
                add_qt(1, 2, "qt12", [(5, 5), (5, 7), (5, 9), (5, 11)])
                add_qt(1, 3, "qt13", [(6, 5), (6, 7), (6, 9), (6, 11)])

                # ---- attention window -------------------------------------
                def attention(m, ig, w, drain=False, pre_norm_needs=()):
                    need(f"qt{m}{ig}" if (m, ig) != (0, 0) else "qt00")
                    vA = slice(2 * m * (DH + 1), 2 * m * (DH + 1) + DH + 1)
                    vB = slice((2 * m + 1) * (DH + 1),
                               (2 * m + 1) * (DH + 1) + DH + 1)
                    oA = pb.tile([128, NI, DH + 1], f32, name="oA", tag="oA",
                                 bufs=1)
                    oB = pb.tile([128, NI, DH + 1], f32, name="oB", tag="oB",
                                 bufs=1)
                    onA = op.tile([128, NI, DH], fp16, name="onA", tag="on",
                                  bufs=4)
                    onB = op.tile([128, NI, DH], fp16, name="onB", tag="on",
                                  bufs=4)
                    es = [None] * NT

                    def qk(j):
                        if m == 0:
                            need(f"kt0p{j}")
                        else:
                            need(f"kt{m}c{j // 4}")
                        jsl = slice(j * 128, (j + 1) * 128)
                        s_ps = pb.tile([128, 1024], f32, name="s_ps",
                                       tag="s_ps", bufs=2)
                        nc.tensor.matmul(s_ps[:, 0:512], kt[m][0:64, jsl],
                                         qt[m][ig][0:64, :],
                                         start=True, stop=True)
                        nc.tensor.matmul(s_ps[:, 512:1024],
                                         kt[m][64:128, jsl],
                                         qt[m][ig][64:128, :],
                                         start=True, stop=True)
                        e = ep.tile([128, 1024], fp16, name="e_sb")
                        nc.scalar.activation(out=e[:], in_=s_ps[:], func=Exp)
                        es[j] = e

                    def pv(j):
                        need(f"v{m}_{j}")
                        e = es[j]
                        # one accumulation group per O bank: start marks the
                        # whole 2KB zero region, so only the first matmul of
                        # the window starts and only the last stops
                        for t in range(NI):
                            mm = nc.tensor.matmul(
                                oA[:, t, :], e[:, t * 128:(t + 1) * 128],
                                vsb[j][:, vA],
                                start=(j == 0 and t == 0),
                                stop=(j == NT - 1 and t == NI - 1))
                            if j == 0 and t == 0 and state.get("norm_last"):
                                # the bank-claiming start must wait for the
                                # previous window's normalization reads (the
                                # region-based tracker only sees subtile 0)
                                add_dep_helper(
                                    mm.ins, state["norm_last"].ins,
                                    reason="O-bank WAR vs prev norm")
                        for t in range(NI):
                            nc.tensor.matmul(
                                oB[:, t, :],
                                e[:, 512 + t * 128:512 + (t + 1) * 128],
                                vsb[j][:, vB],
                                start=(j == 0 and t == 0),
                                stop=(j == NT - 1 and t == NI - 1))

                    rr_box = {}

                    def recips():
                        # O[:, :, 64] holds the softmax row-sums
                        rr = sp.tile([128, 8], f32, name="rr", tag="rr")
                        nc.vector.reciprocal(rr[:, 0:4], oA[:, :, DH])
                        nc.vector.reciprocal(rr[:, 4:8], oB[:, :, DH])
                        rr_box["rr"] = rr

                    def norm_t(t):
                        rr = rr_box["rr"]
                        nc.vector.tensor_scalar_mul(
                            onA[:, t, :], oA[:, t, 0:DH], rr[:, t:t + 1])
                        state["norm_last"] = nc.vector.tensor_scalar_mul(
                            onB[:, t, :], oB[:, t, 0:DH], rr[:, 4 + t:5 + t])

                    def norm():
                        recips()
                        # t descending: subtile 0 is read LAST on the
                        # in-order DVE, and the next window's first PV
                        # matmul (whose bank-claiming start only region-
                        # depends on subtile 0) then transitively waits for
                        # all of this window's normalization reads
                        for t in reversed(range(NI)):
                            norm_t(t)

                    for j in range(NT):
                        qk(j)
                        if j < NT - 1:
                            drip(budget_pops=(2 if drain and j >= 13
                                              else 3),
                                 force=(drain and j >= 13))
                        if j >= LAG:
                            if j == LAG and w >= 1:
                                # previous window's deferred PV tail + norm
                                # must emit before this window reuses the
                                # O-accumulator banks
                                need(f"norm{w - 1}")
                            pv(j - LAG)

                    # tail of the PV stream + normalization either run as
                    # thunks early in the next window (so this window's last
                    # exps overlap the next window's QK stream), or inline
                    # for the final window
                    if drain:
                        for j in range(NT - LAG, NT):
                            pv(j)
                        # this window's norm reuses "on" slots of window
                        # w-2; their transpose readers must emit first
                        for nm in pre_norm_needs:
                            need(nm)
                        recips()
                        return onA, onB, norm_t
                    for i, j in enumerate(range(NT - LAG, NT)):
                        add_thunk(f"pv{w}_{j}", lambda j=j: pv(j),
                                  250.0, (w + 1, 0.1 + 0.4 * i))
                    add_thunk(f"norm{w}", norm, 50.0, (w + 1, 2.0))
                    return onA, onB, None

                # ---- transpose O -> O^T thunks ----------------------------
                def make_transposes(mm, ig, onA, onB, tail=False):
                    ots = [None] * NI

                    def tr(t):
                        T = pb.tile([128, 512], f32, name="T",
                                    tag=("s_ps" if tail else "op"), bufs=2)
                        Tf = T.bitcast(fp16)
                        nc.tensor.transpose(Tf[0:64, 0:128], onA[:, t, :],
                                            ident[:])
                        nc.tensor.transpose(Tf[64:128, 0:128], onB[:, t, :],
                                            ident[:])
                        ot = op.tile([128, 128], fp16, name="ot", tag="ot",
                                     bufs=32)
                        nc.vector.tensor_copy(out=ot[:], in_=Tf[:, 0:128])
                        ots[t] = ot

                    thunks = [(f"tr{mm}_{ig}_{t}", lambda t=t: tr(t), 120.0)
                              for t in range(NI)]
                    return thunks, ots

                # ---- output projection ------------------------------------
                def make_outproj(ig, ots0, ots1, tail=False):
                    out_sb = op.tile([128, 4096], fp16, name="out_sb",
                                     tag="out_sb", bufs=2)

                    def group(gi):
                        jt, dch = gi // 2, gi % 2
                        dsl = slice(dch * 512, (dch + 1) * 512)
                        ptag = "s_ps" if (tail and gi % 2 == 0) else "op"
                        out_ps = pb.tile([128, 512], f32, name="out_ps",
                                         tag=ptag, bufs=2)
                        nc.tensor.matmul(out_ps[:, 0:512], ots0[jt][:],
                                         wo_sb[0][:, dsl],
                                         start=True, stop=False)
                        nc.tensor.matmul(out_ps[:, 0:512], ots1[jt][:],
                                         wo_sb[1][:, dsl],
                                         start=False, stop=True)
                        csl = slice(jt * 1024 + dch * 512,
                                    jt * 1024 + (dch + 1) * 512)
                        if tail and gi % 2 == 0:
                            nc.scalar.copy(out=out_sb[:, csl],
                                           in_=out_ps[:, 0:512])
                        else:
                            nc.vector.tensor_copy(out=out_sb[:, csl],
                                                  in_=out_ps[:, 0:512])

                    def ship(jt0, jt1):
                        part_v = part_d[ig * 512 + jt0 * 128:
                                        ig * 512 + jt1 * 128,
                                        :].rearrange(
                            "(jt p) d -> p jt d", p=128)
                        nc.sync.dma_start(
                            out=part_v,
                            in_=out_sb[:, jt0 * 1024:jt1 * 1024]
                            .rearrange("p (jt d) -> p jt d", jt=jt1 - jt0))

                    def ship_t(jt):
                        rows = slice(ig * 512 + jt * 128,
                                     ig * 512 + (jt + 1) * 128)
                        nc.sync.dma_start(
                            out=part_d[rows, :],
                            in_=out_sb[:, jt * 1024:(jt + 1) * 1024])

                    return group, ship, ship_t

                # ---- main flow --------------------------------------------
                windows = [(0, i) for i in range(NI)] + \
                          [(1, i) for i in range(NI)]
                ots0 = {}
                for w, (m, ig) in enumerate(windows):
                    last = (w == len(windows) - 1)
                    pre = ([f"tr{windows[w - 2][0]}_{windows[w - 2][1]}_{t}"
                            for t in range(NI)] if last else ())
                    onA, onB, norm_t = attention(m, ig, w, drain=last,
                                                 pre_norm_needs=pre)
                    thunks, ots = make_transposes(m, ig, onA, onB,
                                                  tail=last)
                    if m == 0:
                        for t, (nm, fn, cost) in enumerate(thunks):
                            add_thunk(nm, fn, cost, (w + 1, 6 + t))
                        ots0[ig] = ots
                        continue
                    if not last:
                        for t, (nm, fn, cost) in enumerate(thunks):
                            add_thunk(nm, fn, cost, (w + 1, 3.5 + 0.8 * t))
                        group, ship, _ = make_outproj(ig, ots0[ig], ots)
                        for gi in range(8):
                            dl = ((w + 1, 6.5 + 0.9 * gi) if gi < 6
                                  else (w + 2, 1 + 0.8 * (gi - 6)))
                            add_thunk(f"opj{ig}_{gi}",
                                      lambda g=gi, grp=group: grp(g), 430.0,
                                      dl)
                        add_thunk(f"ship{ig}", lambda s=ship: s(0, 4), 0.0,
                                  (w + 2, 2.7))
                    else:
                        while heap:
                            pop_one()
                        group, ship, _ = make_outproj(
                            ig, ots0[ig], ots, tail=True)
                        for t in range(NI):
                            norm_t(t)
                            thunks[t][1]()
                            group(2 * t)
                            group(2 * t + 1)
                            if t % 2 == 1:
                                ship(t - 1, t + 1)


def _get_nc():
    if "nc" not in _CACHE:
        _CACHE["nc"] = build_program()
    return _CACHE["nc"]


def make_in_maps(x, Wq, Wkv, Wo):
    import ml_dtypes

    bf16 = ml_dtypes.bfloat16
    scale = DH ** -0.5
    x = np.asarray(x, dtype=np.float32)
    Wq = np.asarray(Wq, dtype=np.float32)
    Wkv = np.asarray(Wkv, dtype=np.float32)
    Wo = np.asarray(Wo, dtype=np.float32)
    xt = [np.ascontiguousarray(x[b].T.astype(bf16)) for b in range(B)]
    in_maps = []
    for c in range(NCORES):
        b, hg = c // HPG, c % HPG
        sl = slice(hg * DGRP, (hg + 1) * DGRP)
        in_maps.append({
            "xt": xt[b],
            "wq": np.ascontiguousarray((Wq[:, sl] * scale).astype(bf16)),
            "wk": np.ascontiguousarray(
                Wkv[:, hg * DGRP:(hg + 1) * DGRP].astype(bf16)),
            "wv": np.ascontiguousarray(
                Wkv[:, DIM + hg * DGRP:DIM + (hg + 1) * DGRP].astype(bf16)),
            "wo": np.ascontiguousarray(Wo[sl, :].astype(np.float16)),
        })
    return in_maps


def combine_outputs(results, bo):
    out = np.zeros((B, N, DIM), dtype=np.float32)
    for c in range(NCORES):
        out[c // HPG] += results[c]["part"]
    out += np.asarray(bo, dtype=np.float32)
    return out


def kernel(x, mask, Wq, Wkv, Wo, bo):
    from concourse.bass_utils import run_bass_kernel_spmd

    nc = _get_nc()
    in_maps = make_in_maps(x, Wq, Wkv, Wo)
    res = run_bass_kernel_spmd(nc, in_maps, list(range(NCORES)))
    return combine_outputs(res.results, bo)


# revision 71
# speedup vs baseline: 1.2007x; 1.0070x over previous
"""Trainium2 Bass kernel for nn_Attention_24008867185039.

Reference computation (B=2, N=2048, DIM=1024, 16 heads x 64):
    q = x @ Wq ; k, v = split(x @ Wkv) ; per-head softmax(q k^T / sqrt(64)) v
    out = attn_out @ Wo + bo
(mask is all-ones per the problem spec, so masking is a no-op.)

Sharding (8 cores): data-parallel over batch (2) x tensor-parallel over 4
head-groups of 4 heads. Host sums the 4 Wo partials per batch, adds bias.

Cost-model shape (per core): the scalar engine's exp stream (128 ops of
[128,1024], ~137us) and the PE stream (~142us) are co-critical.  PE work is
minimized by exploiting that matmul cost is charged per MOVING row only
(stationary loads are free):
  - QK^T: stationary K-tile [64d, 128keys], moving Q [64d, 512q] -> S^T
    [128 keys, 512 q] per (head, j).  (d=64 makes 50% PE util unavoidable.)
  - PV: stationary exp-tile [128 keys, 128 q], moving V [128 keys, 65]
    (64 dims + a ones column that yields the softmax row-sum for free) ->
    O [128 q, 65] accumulated over the 16 key tiles.  65 moving rows per
    matmul instead of 512 halves the attention-PV cost vs the naive layout.
  - O lands as [q, d']: normalization is a native per-partition
    reciprocal + tensor_scalar multiply, then a PE transpose (128 rows)
    rebuilds O^T [d', q] for the (full-util) output projection.
Projections and the output projection are at the full-utilization floor.

Precision: x/Wq/Wkv load as bf16; S^T in f32 PSUM; exp/V/O^T/Wo in fp16
(same PE rate as bf16, 8x finer mantissa).

Schedule: dummy matmuls ramp the PE p-state while the first x/weight DMAs
land (all DMA transfers serialize on one engine pool in the cost model);
a minimal prefix (K^T cols 0:128 of pair0 + Q^T(pair0,chunk0)) starts the
exp stream ~12us in.  Every other projection chain, V tile, transpose and
output-projection group is a deadline-sorted thunk dripped through the
attention windows' PE slack under a credit scheduler (~440ns of drip
budget per exp), with explicit need() guards enforcing producer-before-
consumer emission order.  PV runs LAG=8 key-tiles behind QK; each
window's PV tail + normalization are deferred into the next window so the
exp stream never waits at window boundaries.  PSUM accumulation groups
are bank-granular (start marks the whole 2KB zero region), so each O bank
carries one group per window, and the next window's bank-claiming start
takes an explicit dep on the previous normalization's last DVE read.
PSUM: s 2x[128,1024] + O-accum 2x[128,4x65] + a shared 2-bank ring for
proj chains / transposes / outproj tiles = 8 banks.
"""

import sys

sys.path.insert(0, "/opt/trn_rl_repo")

import numpy as np

B, N, DIM, HEADS, DH = 2, 2048, 1024, 16, 64
HPG = 4                 # heads per core (head group)
DGRP = HPG * DH         # 256: per-core slice of the inner dim
NCORES = 8
KT = DIM // 128         # 8 contraction tiles for projections
NT = N // 128           # 16 sequence tiles of 128
NI = N // 512           # 4 query chunks of 512
MT = DGRP // 128        # 2 head-pair tiles per core
LAG = 8                 # PV trails QK by this many key tiles
WARMUP = 36             # PE p-state ramp matmuls during the first DMAs
SLACK_NS = 440          # drip budget granted per exp op

_CACHE = {}


def build_program(repeats=1):
    import concourse.mybir as mybir
    import concourse.tile as tile
    from concourse import bacc

    f32 = mybir.dt.float32
    bf16 = mybir.dt.bfloat16
    fp16 = mybir.dt.float16

    nc = bacc.Bacc("TRN2", target_bir_lowering=False, debug=False,
                   num_devices=NCORES)

    xt_d = nc.dram_tensor("xt", [DIM, N], bf16, kind="ExternalInput").ap()
    wq_d = nc.dram_tensor("wq", [DIM, DGRP], bf16, kind="ExternalInput").ap()
    wk_d = nc.dram_tensor("wk", [DIM, DGRP], bf16, kind="ExternalInput").ap()
    wv_d = nc.dram_tensor("wv", [DIM, DGRP], bf16, kind="ExternalInput").ap()
    wo_d = nc.dram_tensor("wo", [DGRP, DIM], fp16, kind="ExternalInput").ap()
    part_d = nc.dram_tensor("part", [N, DIM], fp16,
                            kind="ExternalOutput").ap()

    with tile.TileContext(nc) as tc:
        for rep in range(repeats):
            _emit_body(nc, tc, xt_d, wq_d, wk_d, wv_d, wo_d, part_d,
                       tag=f"r{rep}")

    nc.compile()
    return nc


def _emit_body(nc, tc, xt_d, wq_d, wk_d, wv_d, wo_d, part_d, tag):
    import concourse.mybir as mybir
    from concourse.masks import make_identity
    from concourse.tile_rust import add_dep_helper

    f32 = mybir.dt.float32
    bf16 = mybir.dt.bfloat16
    fp16 = mybir.dt.float16
    Exp = mybir.ActivationFunctionType.Exp

    xt_t = xt_d.rearrange("(t p) n -> p t n", p=128)    # [128, KT, N]
    wq_t = wq_d.rearrange("(t p) d -> p t d", p=128)    # [128, KT, DGRP]
    wk_t = wk_d.rearrange("(t p) d -> p t d", p=128)
    wv_t = wv_d.rearrange("(t p) d -> p t d", p=128)
    wo_t = wo_d.rearrange("(t p) d -> t p d", p=128)    # [MT, 128, DIM]

    with nc.allow_low_precision(reason="fp16/bf16 rounding is intentional"):
        with tc.tile_pool(name=f"pp{tag}", bufs=1) as pp, \
             tc.tile_pool(name=f"sp{tag}", bufs=4) as sp, \
             tc.tile_pool(name=f"ep{tag}", bufs=16) as ep, \
             tc.tile_pool(name=f"osb{tag}", bufs=2) as op:

            # ---- persistent SBUF ------------------------------------------
            # x^T lives as one tile per 512-token chunk so consumers only
            # depend on the DMA that actually feeds them
            xt_sb = [pp.tile([128, KT, 512], bf16, name=f"xt_sb{c}")
                     for c in range(NI)]
            wq_sb = pp.tile([128, KT, DGRP], bf16, name="wq_sb")
            wk_sb = pp.tile([128, KT, DGRP], bf16, name="wk_sb")
            wv_sb = pp.tile([128, KT, DGRP], bf16, name="wv_sb")
            wo_sb = [pp.tile([128, DIM], fp16, name=f"wo_sb{m}")
                     for m in range(MT)]
            qt = [[pp.tile([128, 512], fp16, name=f"qt{m}_{i}")
                   for i in range(NI)] for m in range(MT)]
            kt = [pp.tile([128, N], fp16, name=f"kt{m}") for m in range(MT)]
            # V with a ones column per head: [128, 4 heads x (64 d + 1)]
            vsb = [pp.tile([128, HPG * (DH + 1)], fp16, name=f"vsb{j}")
                   for j in range(NT)]
            ident = pp.tile([128, 128], fp16, name="ident")
            # warmup operand, independent of ident so the PE ramp matmuls
            # don't wait on the Pool engine's startup memset queue
            wrm = pp.tile([128, 128], fp16, name="wrm")
            nc.vector.memset(wrm[:], 0.125)
            # touch Exp once so the activation table is resident before the
            # first real exp (the lazy load would otherwise delay it); uses
            # its own tiny tile so the warmup matmuls don't wait on it
            pre = pp.tile([1, 2], fp16, name="pre")
            nc.vector.memset(pre[:], 0.1)
            nc.scalar.activation(out=pre[0:1, 0:1], in_=pre[0:1, 1:2],
                                 func=mybir.ActivationFunctionType.Exp)
            make_identity(nc, ident[:])

            # ---- DMAs (ordered by first need) -----------------------------
            # All transfers serialize on the DMA-engine pool, and runs
            # under 512B get half bandwidth, so weights go as full-width
            # transfers and x's first chunk in two 256-col (512B-run) pieces
            nc.sync.dma_start(out=wk_sb[:], in_=wk_t)
            nc.sync.dma_start(out=xt_sb[0][:, :, 0:256],
                              in_=xt_t[:, :, 0:256])
            nc.sync.dma_start(out=wq_sb[:], in_=wq_t)
            nc.sync.dma_start(out=xt_sb[0][:, :, 256:512],
                              in_=xt_t[:, :, 256:512])
            nc.sync.dma_start(out=wv_sb[:], in_=wv_t)
            nc.sync.dma_start(out=xt_sb[1][:], in_=xt_t[:, :, 512:1024])
            nc.sync.dma_start(out=xt_sb[2][:], in_=xt_t[:, :, 1024:1536])
            nc.sync.dma_start(out=xt_sb[3][:], in_=xt_t[:, :, 1536:2048])
            for m in range(MT):
                nc.sync.dma_start(out=wo_sb[m][:], in_=wo_t[m])

            # ---- thunk scheduler (deadline-ordered drip queue) ------------
            import heapq

            def v3(j):
                return vsb[j].rearrange("p (h c) -> p h c", h=HPG)

            emitted = set()
            heap = []
            state = {"credit": 0.0, "seq": 0}

            def add_thunk(name, fn, cost, deadline):
                state["seq"] += 1
                heapq.heappush(heap, (deadline, state["seq"], name, fn,
                                      cost))

            def pop_one():
                _, _, name, fn, cost = heapq.heappop(heap)
                fn()
                emitted.add(name)
                # debt floor: a forced overdraw stalls the exp stream once;
                # later windows shouldn't keep paying for it
                state["credit"] = max(state["credit"] - cost, -1200.0)

            def drip(budget_pops=3, force=False):
                # cap stops surplus from bursting several projection chains
                # into one slot (which would starve the exp stream)
                state["credit"] = min(state["credit"] + SLACK_NS, 1800.0)
                n = 0
                while heap and n < budget_pops and (
                        force or state["credit"] >= heap[0][4]):
                    pop_one()
                    n += 1

            def need(name):
                while name not in emitted:
                    assert heap, f"thunk {name} was never queued"
                    pop_one()

            # ---- Phase A: warmup + minimal prefix -------------------------
            with tc.tile_pool(name=f"pa{tag}", bufs=1, space="PSUM") as pa:
                scratch = pa.tile([128, 128], f32, name="scratch")
                for _ in range(WARMUP):
                    nc.tensor.matmul(scratch[:], wrm[:], wrm[:],
                                     start=True, stop=True)
                # K^T(pair0) cols 0:128 only — just enough for QK(j=0)
                kc_ps = pa.tile([128, 128], f32, name="kc_ps")
                for k in range(KT):
                    nc.tensor.matmul(kc_ps[:], wk_sb[:, k, 0:128],
                                     xt_sb[0][:, k, 0:128],
                                     start=(k == 0), stop=(k == KT - 1))
                nc.vector.tensor_copy(out=kt[0][:, 0:128], in_=kc_ps[:])
                emitted.add("kt0p0")
                for _ in range(20):
                    nc.tensor.matmul(scratch[:], wrm[:], wrm[:],
                                     start=True, stop=True)
                # Q^T(pair0) chunk0, in two half-chains pipelined against
                # the two x column-piece DMAs
                for h, csl in enumerate((slice(0, 256), slice(256, 512))):
                    q_ps = pa.tile([128, 256], f32, name=f"q_ps{h}")
                    for k in range(KT):
                        nc.tensor.matmul(q_ps[:], wq_sb[:, k, 0:128],
                                         xt_sb[0][:, k, csl],
                                         start=(k == 0), stop=(k == KT - 1))
                    nc.vector.tensor_copy(out=qt[0][0][:, csl], in_=q_ps[:])
                    if h == 0:
                        for _ in range(19):
                            nc.tensor.matmul(scratch[:], wrm[:], wrm[:],
                                             start=True, stop=True)
                emitted.add("qt00")

            # ---- dripped projection thunks --------------------------------
            with tc.tile_pool(name=f"pb{tag}", bufs=2, space="PSUM") as pb:

                def proj_cols(w_sb, msl, ich, csl, dst_copy):
                    p_ps = pb.tile([128, 512], f32, name="p_ps", tag="op",
                                   bufs=2)
                    ncols = csl.stop - csl.start
                    for k in range(KT):
                        nc.tensor.matmul(p_ps[:, 0:ncols],
                                         w_sb[:, k, msl],
                                         xt_sb[ich][:, k, csl],
                                         start=(k == 0), stop=(k == KT - 1))
                    dst_copy(p_ps[:, 0:ncols])

                def kt_cols(m, ich, c0, c1):
                    gsl = slice(ich * 512 + c0, ich * 512 + c1)
                    proj_cols(
                        wk_sb, slice(m * 128, (m + 1) * 128), ich,
                        slice(c0, c1),
                        lambda p: nc.vector.tensor_copy(out=kt[m][:, gsl],
                                                        in_=p))

                def qt_cols(m, ich, c0, c1):
                    proj_cols(
                        wq_sb, slice(m * 128, (m + 1) * 128), ich,
                        slice(c0, c1),
                        lambda p: nc.vector.tensor_copy(
                            out=qt[m][ich][:, c0:c1], in_=p))

                def v_tile(pair, j):
                    csl = slice((j % 4) * 128, (j % 4) * 128 + 128)
                    msl = slice(pair * 128, (pair + 1) * 128)
                    v_ps = pb.tile([128, 512], f32, name="vp_ps", tag="op",
                                   bufs=2)
                    for k in range(KT):
                        nc.tensor.matmul(v_ps[:, 0:128],
                                         xt_sb[j // 4][:, k, csl],
                                         wv_sb[:, k, msl],
                                         start=(k == 0), stop=(k == KT - 1))
                    nc.vector.tensor_copy(
                        out=v3(j)[:, 2 * pair:2 * pair + 2, 0:DH],
                        in_=v_ps[:, 0:128].rearrange("p (h c) -> p h c", h=2))
                    nc.gpsimd.memset(
                        v3(j)[:, 2 * pair:2 * pair + 2, DH:DH + 1], 1.0)

                QRT, VCOST = 427.0, 427.0

                def add_qt(m, ich, name, dls):
                    # quarter chains; the full-chunk name goes on the last
                    # quarter so need(name) forces all four
                    for q in range(4):
                        add_thunk(name + ("" if q == 3 else f"_{q}"),
                                  lambda q=q: qt_cols(m, ich, q * 128,
                                                      (q + 1) * 128),
                                  QRT, dls[q])

                # prefix extension: kt0 key-pieces 1-3 run in the PE gaps
                # while the first DMAs land (emitted by the code below, not
                # dripped)
                def kt0_piece(j):
                    kt_cols(0, j // 4, (j % 4) * 128, (j % 4) * 128 + 128)

                for j in (1, 2, 3):
                    kt0_piece(j)
                    emitted.add(f"kt0p{j}")
                # pair-0 K^T in 128-col pieces so the forced drip in the
                # first window stays fine-grained
                for j in range(4, NT):
                    add_thunk(f"kt0p{j}", lambda j=j: kt0_piece(j),
                              VCOST, (0, j - 0.5))
                # V tiles whose PV consumer is deferred into the next window
                # spill their deadline there too, interleaved with the
                # deferred PV thunks that consume them
                for j in range(NT):
                    # spilled V tiles must sort BEFORE their deferred-PV
                    # consumer, which must sort before that window's norm
                    dl = (0, j + 7.7) if j <= 7 else (1, 0.35 * (j - 8))
                    add_thunk(f"v0_{j}", lambda j=j: v_tile(0, j), VCOST,
                              dl)
                add_qt(0, 1, "qt01", [(0, 8), (0, 9.7), (0, 11.4),
                                      (0, 13.1)])
                for h in range(2 * NI):
                    w_, o_ = (1, 6) if h < 2 else ((2, 6) if h < 4
                                                   else (3, 1))
                    add_thunk(f"kt1c{h // 2}" + ("" if h % 2 else "a"),
                              lambda h=h: kt_cols(1, h // 2, (h % 2) * 256,
                                                  (h % 2) * 256 + 256),
                              853.0, (w_, o_ + 3 * (h % 2 if h < 4
                                                    else h - 4)))
                add_qt(0, 2, "qt02", [(1, 7), (1, 9), (1, 11), (1, 13)])
                for j in range(0, 8):
                    add_thunk(f"v1_{j}", lambda j=j: v_tile(1, j), VCOST,
                              (2, 4 + 0.9 * j))
                add_qt(0, 3, "qt03", [(2, 7), (2, 9), (2, 11), (2, 13)])
                for j in range(8, NT):
                    add_thunk(f"v1_{j}", lambda j=j: v_tile(1, j), VCOST,
                              (5, 0.35 * (j - 8)))
                add_qt(1, 0, "qt10", [(3, 5), (3, 7.5), (3, 10), (3, 12.5)])
                add_qt(1, 1, "qt11", [(4, 5), (4, 7), (4, 9), (4, 11)])
                add_qt(1, 2, "qt12", [(5, 5), (5, 7), (5, 9), (5, 11)])
                add_qt(1, 3, "qt13", [(6, 5), (6, 7), (6, 9), (6, 11)])

                # ---- attention window -------------------------------------
                def attention(m, ig, w, drain=False, pre_norm_needs=()):
                    need(f"qt{m}{ig}" if (m, ig) != (0, 0) else "qt00")
                    vA = slice(2 * m * (DH + 1), 2 * m * (DH + 1) + DH + 1)
                    vB = slice((2 * m + 1) * (DH + 1),
                               (2 * m + 1) * (DH + 1) + DH + 1)
                    oA = pb.tile([128, NI, DH + 1], f32, name="oA", tag="oA",
                                 bufs=1)
                    oB = pb.tile([128, NI, DH + 1], f32, name="oB", tag="oB",
                                 bufs=1)
                    onA = op.tile([128, NI, DH], fp16, name="onA", tag="on",
                                  bufs=4)
                    onB = op.tile([128, NI, DH], fp16, name="onB", tag="on",
                                  bufs=4)
                    es = [None] * NT

                    def qk(j):
                        if m == 0:
                            need(f"kt0p{j}")
                        else:
                            need(f"kt{m}c{j // 4}")
                        jsl = slice(j * 128, (j + 1) * 128)
                        s_ps = pb.tile([128, 1024], f32, name="s_ps",
                                       tag="s_ps", bufs=2)
                        nc.tensor.matmul(s_ps[:, 0:512], kt[m][0:64, jsl],
                                         qt[m][ig][0:64, :],
                                         start=True, stop=True)
                        nc.tensor.matmul(s_ps[:, 512:1024],
                                         kt[m][64:128, jsl],
                                         qt[m][ig][64:128, :],
                                         start=True, stop=True)
                        e = ep.tile([128, 1024], fp16, name="e_sb")
                        nc.scalar.activation(out=e[:], in_=s_ps[:], func=Exp)
                        es[j] = e

                    def pv(j):
                        need(f"v{m}_{j}")
                        e = es[j]
                        # one accumulation group per O bank: start marks the
                        # whole 2KB zero region, so only the first matmul of
                        # the window starts and only the last stops
                        for t in range(NI):
                            mm = nc.tensor.matmul(
                                oA[:, t, :], e[:, t * 128:(t + 1) * 128],
                                vsb[j][:, vA],
                                start=(j == 0 and t == 0),
                                stop=(j == NT - 1 and t == NI - 1))
                            if j == 0 and t == 0 and state.get("norm_last"):
                                # the bank-claiming start must wait for the
                                # previous window's normalization reads (the
                                # region-based tracker only sees subtile 0)
                                add_dep_helper(
                                    mm.ins, state["norm_last"].ins,
                                    reason="O-bank WAR vs prev norm")
                        for t in range(NI):
                            nc.tensor.matmul(
                                oB[:, t, :],
                                e[:, 512 + t * 128:512 + (t + 1) * 128],
                                vsb[j][:, vB],
                                start=(j == 0 and t == 0),
                                stop=(j == NT - 1 and t == NI - 1))

                    rr_box = {}

                    def recips():
                        # O[:, :, 64] holds the softmax row-sums
                        rr = sp.tile([128, 8], f32, name="rr", tag="rr")
                        nc.vector.reciprocal(rr[:, 0:4], oA[:, :, DH])
                        nc.vector.reciprocal(rr[:, 4:8], oB[:, :, DH])
                        rr_box["rr"] = rr

                    def norm_t(t):
                        rr = rr_box["rr"]
                        nc.vector.tensor_scalar_mul(
                            onA[:, t, :], oA[:, t, 0:DH], rr[:, t:t + 1])
                        state["norm_last"] = nc.vector.tensor_scalar_mul(
                            onB[:, t, :], oB[:, t, 0:DH], rr[:, 4 + t:5 + t])

                    def norm():
                        recips()
                        # t descending: subtile 0 is read LAST on the
                        # in-order DVE, and the next window's first PV
                        # matmul (whose bank-claiming start only region-
                        # depends on subtile 0) then transitively waits for
                        # all of this window's normalization reads
                        for t in reversed(range(NI)):
                            norm_t(t)

                    for j in range(NT):
                        if j == LAG and w >= 1:
                            # previous window's deferred PV tail + norm must
                            # emit before this window reuses the O banks or
                            # their exp tiles' SBUF slots
                            need(f"norm{w - 1}")
                        qk(j)
                        if j < NT - 1:
                            drip(budget_pops=(2 if drain and j >= 13
                                              else 3),
                                 force=(drain and j >= 13))
                        if j >= LAG:
                            pv(j - LAG)

                    # tail of the PV stream + normalization either run as
                    # thunks early in the next window (so this window's last
                    # exps overlap the next window's QK stream), or inline
                    # for the final window
                    if drain:
                        for j in range(NT - LAG, NT):
                            pv(j)
                        # this window's norm reuses "on" slots of window
                        # w-2; their transpose readers must emit first
                        for nm in pre_norm_needs:
                            need(nm)
                        recips()
                        return onA, onB, norm_t
                    for i, j in enumerate(range(NT - LAG, NT)):
                        add_thunk(f"pv{w}_{j}", lambda j=j: pv(j),
                                  250.0, (w + 1, 0.1 + 0.35 * i))
                    add_thunk(f"norm{w}", norm, 50.0, (w + 1, 3.0))
                    return onA, onB, None

                # ---- transpose O -> O^T thunks ----------------------------
                def make_transposes(mm, ig, onA, onB, tail=False):
                    ots = [None] * NI

                    def tr(t):
                        T = pb.tile([128, 512], f32, name="T",
                                    tag=("s_ps" if tail else "op"), bufs=2)
                        Tf = T.bitcast(fp16)
                        nc.tensor.transpose(Tf[0:64, 0:128], onA[:, t, :],
                                            ident[:])
                        nc.tensor.transpose(Tf[64:128, 0:128], onB[:, t, :],
                                            ident[:])
                        ot = op.tile([128, 128], fp16, name="ot", tag="ot",
                                     bufs=32)
                        nc.vector.tensor_copy(out=ot[:], in_=Tf[:, 0:128])
                        ots[t] = ot

                    thunks = [(f"tr{mm}_{ig}_{t}", lambda t=t: tr(t), 120.0)
                              for t in range(NI)]
                    return thunks, ots

                # ---- output projection ------------------------------------
                def make_outproj(ig, ots0, ots1, tail=False):
                    out_sb = op.tile([128, 4096], fp16, name="out_sb",
                                     tag="out_sb", bufs=2)

                    def group(gi):
                        jt, dch = gi // 2, gi % 2
                        dsl = slice(dch * 512, (dch + 1) * 512)
                        ptag = "s_ps" if (tail and gi % 2 == 0) else "op"
                        out_ps = pb.tile([128, 512], f32, name="out_ps",
                                         tag=ptag, bufs=2)
                        nc.tensor.matmul(out_ps[:, 0:512], ots0[jt][:],
                                         wo_sb[0][:, dsl],
                                         start=True, stop=False)
                        nc.tensor.matmul(out_ps[:, 0:512], ots1[jt][:],
                                         wo_sb[1][:, dsl],
                                         start=False, stop=True)
                        csl = slice(jt * 1024 + dch * 512,
                                    jt * 1024 + (dch + 1) * 512)
                        if tail and gi % 2 == 0:
                            nc.scalar.copy(out=out_sb[:, csl],
                                           in_=out_ps[:, 0:512])
                        else:
                            nc.vector.tensor_copy(out=out_sb[:, csl],
                                                  in_=out_ps[:, 0:512])

                    def ship(jt0, jt1):
                        part_v = part_d[ig * 512 + jt0 * 128:
                                        ig * 512 + jt1 * 128,
                                        :].rearrange(
                            "(jt p) d -> p jt d", p=128)
                        nc.sync.dma_start(
                            out=part_v,
                            in_=out_sb[:, jt0 * 1024:jt1 * 1024]
                            .rearrange("p (jt d) -> p jt d", jt=jt1 - jt0))

                    def ship_t(jt):
                        rows = slice(ig * 512 + jt * 128,
                                     ig * 512 + (jt + 1) * 128)
                        nc.sync.dma_start(
                            out=part_d[rows, :],
                            in_=out_sb[:, jt * 1024:(jt + 1) * 1024])

                    return group, ship, ship_t

                # ---- main flow --------------------------------------------
                windows = [(0, i) for i in range(NI)] + \
                          [(1, i) for i in range(NI)]
                ots0 = {}
                for w, (m, ig) in enumerate(windows):
                    last = (w == len(windows) - 1)
                    pre = ([f"tr{windows[w - 2][0]}_{windows[w - 2][1]}_{t}"
                            for t in range(NI)] if last else ())
                    onA, onB, norm_t = attention(m, ig, w, drain=last,
                                                 pre_norm_needs=pre)
                    thunks, ots = make_transposes(m, ig, onA, onB,
                                                  tail=last)
                    if m == 0:
                        for t, (nm, fn, cost) in enumerate(thunks):
                            add_thunk(nm, fn, cost, (w + 1, 6 + t))
                        ots0[ig] = ots
                        continue
                    if not last:
                        for t, (nm, fn, cost) in enumerate(thunks):
                            add_thunk(nm, fn, cost, (w + 1, 3.5 + 0.8 * t))
                        group, ship, _ = make_outproj(ig, ots0[ig], ots)
                        for gi in range(8):
                            if ig < NI - 2:
                                dl = ((w + 1, 6.5 + 0.9 * gi) if gi < 6
                                      else (w + 2, 1 + 0.8 * (gi - 6)))
                            else:
                                # the last deferred chunk must fully drain
                                # inside the final window
                                dl = (w + 1, 6.5 + 0.85 * gi)
                            add_thunk(f"opj{ig}_{gi}",
                                      lambda g=gi, grp=group: grp(g), 430.0,
                                      dl)
                        add_thunk(f"ship{ig}", lambda s=ship: s(0, 4), 0.0,
                                  (w + 2, 2.7) if ig < NI - 2
                                  else (w + 1, 13.8))
                    else:
                        while heap:
                            pop_one()
                        group, ship, _ = make_outproj(
                            ig, ots0[ig], ots, tail=True)
                        for t in range(NI):
                            norm_t(t)
                            thunks[t][1]()
                            group(2 * t)
                            group(2 * t + 1)
                            if t % 2 == 1:
                                ship(t - 1, t + 1)


def _get_nc():
    if "nc" not in _CACHE:
        _CACHE["nc"] = build_program()
    return _CACHE["nc"]


def make_in_maps(x, Wq, Wkv, Wo):
    import ml_dtypes

    bf16 = ml_dtypes.bfloat16
    scale = DH ** -0.5
    x = np.asarray(x, dtype=np.float32)
    Wq = np.asarray(Wq, dtype=np.float32)
    Wkv = np.asarray(Wkv, dtype=np.float32)
    Wo = np.asarray(Wo, dtype=np.float32)
    xt = [np.ascontiguousarray(x[b].T.astype(bf16)) for b in range(B)]
    in_maps = []
    for c in range(NCORES):
        b, hg = c // HPG, c % HPG
        sl = slice(hg * DGRP, (hg + 1) * DGRP)
        in_maps.append({
            "xt": xt[b],
            "wq": np.ascontiguousarray((Wq[:, sl] * scale).astype(bf16)),
            "wk": np.ascontiguousarray(
                Wkv[:, hg * DGRP:(hg + 1) * DGRP].astype(bf16)),
            "wv": np.ascontiguousarray(
                Wkv[:, DIM + hg * DGRP:DIM + (hg + 1) * DGRP].astype(bf16)),
            "wo": np.ascontiguousarray(Wo[sl, :].astype(np.float16)),
        })
    return in_maps


def combine_outputs(results, bo):
    out = np.zeros((B, N, DIM), dtype=np.float32)
    for c in range(NCORES):
        out[c // HPG] += results[c]["part"]
    out += np.asarray(bo, dtype=np.float32)
    return out


def kernel(x, mask, Wq, Wkv, Wo, bo):
    from concourse.bass_utils import run_bass_kernel_spmd

    nc = _get_nc()
    in_maps = make_in_maps(x, Wq, Wkv, Wo)
    res = run_bass_kernel_spmd(nc, in_maps, list(range(NCORES)))
    return combine_outputs(res.results, bo)


# revision 75
# speedup vs baseline: 1.2110x; 1.0085x over previous
"""Trainium2 Bass kernel for nn_Attention_24008867185039.

Reference computation (B=2, N=2048, DIM=1024, 16 heads x 64):
    q = x @ Wq ; k, v = split(x @ Wkv) ; per-head softmax(q k^T / sqrt(64)) v
    out = attn_out @ Wo + bo
(mask is all-ones per the problem spec, so masking is a no-op.)

Sharding (8 cores): data-parallel over batch (2) x tensor-parallel over 4
head-groups of 4 heads. Host sums the 4 Wo partials per batch, adds bias.

Cost-model shape (per core): the scalar engine's exp stream (128 ops of
[128,1024], ~137us) and the PE stream (~142us) are co-critical.  PE work is
minimized by exploiting that matmul cost is charged per MOVING row only
(stationary loads are free):
  - QK^T: stationary K-tile [64d, 128keys], moving Q [64d, 512q] -> S^T
    [128 keys, 512 q] per (head, j).  (d=64 makes 50% PE util unavoidable.)
  - PV: stationary exp-tile [128 keys, 128 q], moving V [128 keys, 65]
    (64 dims + a ones column that yields the softmax row-sum for free) ->
    O [128 q, 65] accumulated over the 16 key tiles.  65 moving rows per
    matmul instead of 512 halves the attention-PV cost vs the naive layout.
  - O lands as [q, d']: normalization is a native per-partition
    reciprocal + tensor_scalar multiply, then a PE transpose (128 rows)
    rebuilds O^T [d', q] for the (full-util) output projection.
Projections and the output projection are at the full-utilization floor.

Precision: x/Wq/Wkv load as bf16; S^T in f32 PSUM; exp/V/O^T/Wo in fp16
(same PE rate as bf16, 8x finer mantissa).

Schedule: dummy matmuls ramp the PE p-state while the first x/weight DMAs
land (all DMA transfers serialize on one engine pool in the cost model);
a minimal prefix (K^T cols 0:128 of pair0 + Q^T(pair0,chunk0)) starts the
exp stream ~12us in.  Every other projection chain, V tile, transpose and
output-projection group is a deadline-sorted thunk dripped through the
attention windows' PE slack under a credit scheduler (~440ns of drip
budget per exp), with explicit need() guards enforcing producer-before-
consumer emission order.  PV runs LAG=8 key-tiles behind QK; each
window's PV tail + normalization are deferred into the next window so the
exp stream never waits at window boundaries.  PSUM accumulation groups
are bank-granular (start marks the whole 2KB zero region), so each O bank
carries one group per window, and the next window's bank-claiming start
takes an explicit dep on the previous normalization's last DVE read.
PSUM: s 2x[128,1024] + O-accum 2x[128,4x65] + a shared 2-bank ring for
proj chains / transposes / outproj tiles = 8 banks.
"""

import sys

sys.path.insert(0, "/opt/trn_rl_repo")

import numpy as np

B, N, DIM, HEADS, DH = 2, 2048, 1024, 16, 64
HPG = 4                 # heads per core (head group)
DGRP = HPG * DH         # 256: per-core slice of the inner dim
NCORES = 8
KT = DIM // 128         # 8 contraction tiles for projections
NT = N // 128           # 16 sequence tiles of 128
NI = N // 512           # 4 query chunks of 512
MT = DGRP // 128        # 2 head-pair tiles per core
LAG = 8                 # PV trails QK by this many key tiles
WARMUP = 36             # PE p-state ramp matmuls during the first DMAs
SLACK_NS = 440          # drip budget granted per exp op

_CACHE = {}


def build_program(repeats=1):
    import concourse.mybir as mybir
    import concourse.tile as tile
    from concourse import bacc

    f32 = mybir.dt.float32
    bf16 = mybir.dt.bfloat16
    fp16 = mybir.dt.float16

    nc = bacc.Bacc("TRN2", target_bir_lowering=False, debug=False,
                   num_devices=NCORES)

    xt_d = nc.dram_tensor("xt", [DIM, N], bf16, kind="ExternalInput").ap()
    wq_d = nc.dram_tensor("wq", [DIM, DGRP], bf16, kind="ExternalInput").ap()
    wk_d = nc.dram_tensor("wk", [DIM, DGRP], bf16, kind="ExternalInput").ap()
    wv_d = nc.dram_tensor("wv", [DIM, DGRP], bf16, kind="ExternalInput").ap()
    wo_d = nc.dram_tensor("wo", [DGRP, DIM], fp16, kind="ExternalInput").ap()
    part_d = nc.dram_tensor("part", [N, DIM], fp16,
                            kind="ExternalOutput").ap()

    with tile.TileContext(nc) as tc:
        for rep in range(repeats):
            _emit_body(nc, tc, xt_d, wq_d, wk_d, wv_d, wo_d, part_d,
                       tag=f"r{rep}")

    nc.compile()
    return nc


def _emit_body(nc, tc, xt_d, wq_d, wk_d, wv_d, wo_d, part_d, tag):
    import concourse.mybir as mybir
    from concourse.masks import make_identity
    from concourse.tile_rust import add_dep_helper

    f32 = mybir.dt.float32
    bf16 = mybir.dt.bfloat16
    fp16 = mybir.dt.float16
    Exp = mybir.ActivationFunctionType.Exp

    xt_t = xt_d.rearrange("(t p) n -> p t n", p=128)    # [128, KT, N]
    wq_t = wq_d.rearrange("(t p) d -> p t d", p=128)    # [128, KT, DGRP]
    wk_t = wk_d.rearrange("(t p) d -> p t d", p=128)
    wv_t = wv_d.rearrange("(t p) d -> p t d", p=128)
    wo_t = wo_d.rearrange("(t p) d -> t p d", p=128)    # [MT, 128, DIM]

    with nc.allow_low_precision(reason="fp16/bf16 rounding is intentional"):
        with tc.tile_pool(name=f"pp{tag}", bufs=1) as pp, \
             tc.tile_pool(name=f"sp{tag}", bufs=4) as sp, \
             tc.tile_pool(name=f"ep{tag}", bufs=16) as ep, \
             tc.tile_pool(name=f"osb{tag}", bufs=2) as op:

            # ---- persistent SBUF ------------------------------------------
            # x^T lives as one tile per 512-token chunk so consumers only
            # depend on the DMA that actually feeds them
            xt_sb = [pp.tile([128, KT, 512], bf16, name=f"xt_sb{c}")
                     for c in range(NI)]
            wq_sb = pp.tile([128, KT, DGRP], bf16, name="wq_sb")
            wk_sb = pp.tile([128, KT, DGRP], bf16, name="wk_sb")
            wv_sb = pp.tile([128, KT, DGRP], bf16, name="wv_sb")
            wo_sb = [pp.tile([128, DIM], fp16, name=f"wo_sb{m}")
                     for m in range(MT)]
            qt = [[pp.tile([128, 512], fp16, name=f"qt{m}_{i}")
                   for i in range(NI)] for m in range(MT)]
            kt = [pp.tile([128, N], fp16, name=f"kt{m}") for m in range(MT)]
            # V with a ones column per head: [128, 4 heads x (64 d + 1)]
            vsb = [pp.tile([128, HPG * (DH + 1)], fp16, name=f"vsb{j}")
                   for j in range(NT)]
            ident = pp.tile([128, 128], fp16, name="ident")
            # warmup operand, independent of ident so the PE ramp matmuls
            # don't wait on the Pool engine's startup memset queue
            wrm = pp.tile([128, 128], fp16, name="wrm")
            nc.vector.memset(wrm[:], 0.125)
            # touch Exp once so the activation table is resident before the
            # first real exp (the lazy load would otherwise delay it); uses
            # its own tiny tile so the warmup matmuls don't wait on it
            pre = pp.tile([1, 2], fp16, name="pre")
            nc.vector.memset(pre[:], 0.1)
            nc.scalar.activation(out=pre[0:1, 0:1], in_=pre[0:1, 1:2],
                                 func=mybir.ActivationFunctionType.Exp)
            make_identity(nc, ident[:])

            # ---- DMAs (ordered by first need) -----------------------------
            # All transfers serialize on the DMA-engine pool, and runs
            # under 512B get half bandwidth, so weights go as full-width
            # transfers and x's first chunk in two 256-col (512B-run) pieces
            nc.sync.dma_start(out=wk_sb[:], in_=wk_t)
            nc.sync.dma_start(out=xt_sb[0][:, :, 0:256],
                              in_=xt_t[:, :, 0:256])
            nc.sync.dma_start(out=wq_sb[:], in_=wq_t)
            nc.sync.dma_start(out=xt_sb[0][:, :, 256:512],
                              in_=xt_t[:, :, 256:512])
            nc.sync.dma_start(out=wv_sb[:], in_=wv_t)
            nc.sync.dma_start(out=xt_sb[1][:], in_=xt_t[:, :, 512:1024])
            nc.sync.dma_start(out=xt_sb[2][:], in_=xt_t[:, :, 1024:1536])
            nc.sync.dma_start(out=xt_sb[3][:], in_=xt_t[:, :, 1536:2048])
            for m in range(MT):
                nc.sync.dma_start(out=wo_sb[m][:], in_=wo_t[m])

            # ---- thunk scheduler (deadline-ordered drip queue) ------------
            import heapq

            def v3(j):
                return vsb[j].rearrange("p (h c) -> p h c", h=HPG)

            emitted = set()
            heap = []
            state = {"credit": 0.0, "seq": 0}

            def add_thunk(name, fn, cost, deadline):
                state["seq"] += 1
                heapq.heappush(heap, (deadline, state["seq"], name, fn,
                                      cost))

            def pop_one():
                _, _, name, fn, cost = heapq.heappop(heap)
                fn()
                emitted.add(name)
                # debt floor: a forced overdraw stalls the exp stream once;
                # later windows shouldn't keep paying for it
                state["credit"] = max(state["credit"] - cost, -1200.0)

            def drip(budget_pops=3, force=False):
                # cap stops surplus from bursting several projection chains
                # into one slot (which would starve the exp stream)
                state["credit"] = min(state["credit"] + SLACK_NS, 1800.0)
                n = 0
                while heap and n < budget_pops and (
                        force or state["credit"] >= heap[0][4]):
                    pop_one()
                    n += 1

            def need(name):
                while name not in emitted:
                    assert heap, f"thunk {name} was never queued"
                    pop_one()

            # ---- Phase A: warmup + minimal prefix -------------------------
            with tc.tile_pool(name=f"pa{tag}", bufs=1, space="PSUM") as pa:
                scratch = pa.tile([128, 128], f32, name="scratch")
                for _ in range(WARMUP):
                    nc.tensor.matmul(scratch[:], wrm[:], wrm[:],
                                     start=True, stop=True)
                # K^T(pair0) cols 0:128 only — just enough for QK(j=0)
                kc_ps = pa.tile([128, 128], f32, name="kc_ps")
                for k in range(KT):
                    nc.tensor.matmul(kc_ps[:], wk_sb[:, k, 0:128],
                                     xt_sb[0][:, k, 0:128],
                                     start=(k == 0), stop=(k == KT - 1))
                nc.vector.tensor_copy(out=kt[0][:, 0:128], in_=kc_ps[:])
                emitted.add("kt0p0")
                # keys 128:256 need only xt0a too — fill the wq DMA wait
                kp_ps = pa.tile([128, 128], f32, name="kp_ps")
                for k in range(KT):
                    nc.tensor.matmul(kp_ps[:], wk_sb[:, k, 0:128],
                                     xt_sb[0][:, k, 128:256],
                                     start=(k == 0), stop=(k == KT - 1))
                nc.vector.tensor_copy(out=kt[0][:, 128:256], in_=kp_ps[:])
                emitted.add("kt0p1")
                for _ in range(8):
                    nc.tensor.matmul(scratch[:], wrm[:], wrm[:],
                                     start=True, stop=True)
                # Q^T(pair0) chunk0, in two half-chains pipelined against
                # the two x column-piece DMAs
                for h, csl in enumerate((slice(0, 256), slice(256, 512))):
                    q_ps = pa.tile([128, 256], f32, name=f"q_ps{h}")
                    for k in range(KT):
                        nc.tensor.matmul(q_ps[:], wq_sb[:, k, 0:128],
                                         xt_sb[0][:, k, csl],
                                         start=(k == 0), stop=(k == KT - 1))
                    nc.vector.tensor_copy(out=qt[0][0][:, csl], in_=q_ps[:])
                    if h == 0:
                        for _ in range(19):
                            nc.tensor.matmul(scratch[:], wrm[:], wrm[:],
                                             start=True, stop=True)
                emitted.add("qt00")

            # ---- dripped projection thunks --------------------------------
            with tc.tile_pool(name=f"pb{tag}", bufs=2, space="PSUM") as pb:

                def proj_cols(w_sb, msl, ich, csl, dst_copy):
                    p_ps = pb.tile([128, 512], f32, name="p_ps", tag="op",
                                   bufs=2)
                    ncols = csl.stop - csl.start
                    for k in range(KT):
                        nc.tensor.matmul(p_ps[:, 0:ncols],
                                         w_sb[:, k, msl],
                                         xt_sb[ich][:, k, csl],
                                         start=(k == 0), stop=(k == KT - 1))
                    dst_copy(p_ps[:, 0:ncols])

                def kt_cols(m, ich, c0, c1):
                    gsl = slice(ich * 512 + c0, ich * 512 + c1)
                    proj_cols(
                        wk_sb, slice(m * 128, (m + 1) * 128), ich,
                        slice(c0, c1),
                        lambda p: nc.vector.tensor_copy(out=kt[m][:, gsl],
                                                        in_=p))

                def qt_cols(m, ich, c0, c1):
                    proj_cols(
                        wq_sb, slice(m * 128, (m + 1) * 128), ich,
                        slice(c0, c1),
                        lambda p: nc.vector.tensor_copy(
                            out=qt[m][ich][:, c0:c1], in_=p))

                def v_tile(pair, j):
                    csl = slice((j % 4) * 128, (j % 4) * 128 + 128)
                    msl = slice(pair * 128, (pair + 1) * 128)
                    v_ps = pb.tile([128, 512], f32, name="vp_ps", tag="op",
                                   bufs=2)
                    for k in range(KT):
                        nc.tensor.matmul(v_ps[:, 0:128],
                                         xt_sb[j // 4][:, k, csl],
                                         wv_sb[:, k, msl],
                                         start=(k == 0), stop=(k == KT - 1))
                    nc.vector.tensor_copy(
                        out=v3(j)[:, 2 * pair:2 * pair + 2, 0:DH],
                        in_=v_ps[:, 0:128].rearrange("p (h c) -> p h c", h=2))
                    nc.gpsimd.memset(
                        v3(j)[:, 2 * pair:2 * pair + 2, DH:DH + 1], 1.0)

                QRT, VCOST = 427.0, 427.0

                def add_qt(m, ich, name, dls):
                    # quarter chains; the full-chunk name goes on the last
                    # quarter so need(name) forces all four
                    for q in range(4):
                        add_thunk(name + ("" if q == 3 else f"_{q}"),
                                  lambda q=q: qt_cols(m, ich, q * 128,
                                                      (q + 1) * 128),
                                  QRT, dls[q])

                # prefix extension: kt0 key-pieces 1-3 run in the PE gaps
                # while the first DMAs land (emitted by the code below, not
                # dripped)
                def kt0_piece(j):
                    kt_cols(0, j // 4, (j % 4) * 128, (j % 4) * 128 + 128)

                for j in (2, 3):
                    add_thunk(f"kt0p{j}", lambda j=j: kt0_piece(j), VCOST,
                              (0, j - 1.8))
                # pair-0 K^T in 128-col pieces so the forced drip in the
                # first window stays fine-grained
                for j in range(4, NT):
                    add_thunk(f"kt0p{j}", lambda j=j: kt0_piece(j),
                              VCOST, (0, j - 0.5))
                # V tiles whose PV consumer is deferred into the next window
                # spill their deadline there too, interleaved with the
                # deferred PV thunks that consume them
                for j in range(NT):
                    # spilled V tiles must sort BEFORE their deferred-PV
                    # consumer, which must sort before that window's norm
                    dl = (0, j + 7.7) if j <= 7 else (1, 0.35 * (j - 8))
                    add_thunk(f"v0_{j}", lambda j=j: v_tile(0, j), VCOST,
                              dl)
                add_qt(0, 1, "qt01", [(0, 8), (0, 9.7), (0, 11.4),
                                      (0, 13.1)])
                for h in range(2 * NI):
                    w_, o_ = (1, 6) if h < 2 else ((2, 6) if h < 4
                                                   else (3, 1))
                    add_thunk(f"kt1c{h // 2}" + ("" if h % 2 else "a"),
                              lambda h=h: kt_cols(1, h // 2, (h % 2) * 256,
                                                  (h % 2) * 256 + 256),
                              853.0, (w_, o_ + 3 * (h % 2 if h < 4
                                                    else h - 4)))
                add_qt(0, 2, "qt02", [(1, 7), (1, 9), (1, 11), (1, 13)])
                for j in range(0, 8):
                    add_thunk(f"v1_{j}", lambda j=j: v_tile(1, j), VCOST,
                              (2, 4 + 0.9 * j))
                add_qt(0, 3, "qt03", [(2, 7), (2, 9), (2, 11), (2, 13)])
                for j in range(8, NT):
                    add_thunk(f"v1_{j}", lambda j=j: v_tile(1, j), VCOST,
                              (5, 0.35 * (j - 8)))
                add_qt(1, 0, "qt10", [(3, 5), (3, 7.5), (3, 10), (3, 12.5)])
                add_qt(1, 1, "qt11", [(4, 5), (4, 7), (4, 9), (4, 11)])
                add_qt(1, 2, "qt12", [(5, 5), (5, 7), (5, 9), (5, 11)])
                add_qt(1, 3, "qt13", [(6, 5), (6, 7), (6, 9), (6, 11)])

                # ---- attention window -------------------------------------
                def attention(m, ig, w, drain=False, pre_norm_needs=()):
                    need(f"qt{m}{ig}" if (m, ig) != (0, 0) else "qt00")
                    vA = slice(2 * m * (DH + 1), 2 * m * (DH + 1) + DH + 1)
                    vB = slice((2 * m + 1) * (DH + 1),
                               (2 * m + 1) * (DH + 1) + DH + 1)
                    oA = pb.tile([128, NI, DH + 1], f32, name="oA", tag="oA",
                                 bufs=1)
                    oB = pb.tile([128, NI, DH + 1], f32, name="oB", tag="oB",
                                 bufs=1)
                    onA = op.tile([128, NI, DH], fp16, name="onA", tag="on",
                                  bufs=4)
                    onB = op.tile([128, NI, DH], fp16, name="onB", tag="on",
                                  bufs=4)
                    es = [None] * NT

                    def qk(j):
                        if m == 0:
                            need(f"kt0p{j}")
                        else:
                            need(f"kt{m}c{j // 4}")
                        jsl = slice(j * 128, (j + 1) * 128)
                        s_ps = pb.tile([128, 1024], f32, name="s_ps",
                                       tag="s_ps", bufs=2)
                        nc.tensor.matmul(s_ps[:, 0:512], kt[m][0:64, jsl],
                                         qt[m][ig][0:64, :],
                                         start=True, stop=True)
                        nc.tensor.matmul(s_ps[:, 512:1024],
                                         kt[m][64:128, jsl],
                                         qt[m][ig][64:128, :],
                                         start=True, stop=True)
                        e = ep.tile([128, 1024], fp16, name="e_sb")
                        nc.scalar.activation(out=e[:], in_=s_ps[:], func=Exp)
                        es[j] = e

                    def pv(j):
                        need(f"v{m}_{j}")
                        e = es[j]
                        # one accumulation group per O bank: start marks the
                        # whole 2KB zero region, so only the first matmul of
                        # the window starts and only the last stops
                        for t in range(NI):
                            mm = nc.tensor.matmul(
                                oA[:, t, :], e[:, t * 128:(t + 1) * 128],
                                vsb[j][:, vA],
                                start=(j == 0 and t == 0),
                                stop=(j == NT - 1 and t == NI - 1))
                            if j == 0 and t == 0 and state.get("norm_last"):
                                # the bank-claiming start must wait for the
                                # previous window's normalization reads (the
                                # region-based tracker only sees subtile 0)
                                add_dep_helper(
                                    mm.ins, state["norm_last"].ins,
                                    reason="O-bank WAR vs prev norm")
                        for t in range(NI):
                            nc.tensor.matmul(
                                oB[:, t, :],
                                e[:, 512 + t * 128:512 + (t + 1) * 128],
                                vsb[j][:, vB],
                                start=(j == 0 and t == 0),
                                stop=(j == NT - 1 and t == NI - 1))

                    rr_box = {}

                    def recips():
                        # O[:, :, 64] holds the softmax row-sums
                        rr = sp.tile([128, 8], f32, name="rr", tag="rr")
                        nc.vector.reciprocal(rr[:, 0:4], oA[:, :, DH])
                        nc.vector.reciprocal(rr[:, 4:8], oB[:, :, DH])
                        rr_box["rr"] = rr

                    def norm_t(t):
                        rr = rr_box["rr"]
                        nc.vector.tensor_scalar_mul(
                            onA[:, t, :], oA[:, t, 0:DH], rr[:, t:t + 1])
                        state["norm_last"] = nc.vector.tensor_scalar_mul(
                            onB[:, t, :], oB[:, t, 0:DH], rr[:, 4 + t:5 + t])

                    def norm():
                        recips()
                        # t descending: subtile 0 is read LAST on the
                        # in-order DVE, and the next window's first PV
                        # matmul (whose bank-claiming start only region-
                        # depends on subtile 0) then transitively waits for
                        # all of this window's normalization reads
                        for t in reversed(range(NI)):
                            norm_t(t)

                    for j in range(NT):
                        if j == LAG and w >= 1:
                            # previous window's deferred PV tail + norm must
                            # emit before this window reuses the O banks or
                            # their exp tiles' SBUF slots
                            need(f"norm{w - 1}")
                        qk(j)
                        if j < NT - 1:
                            drip(budget_pops=(2 if drain and j >= 13
                                              else 3),
                                 force=(drain and j >= 13))
                        if j >= LAG:
                            pv(j - LAG)

                    # tail of the PV stream + normalization either run as
                    # thunks early in the next window (so this window's last
                    # exps overlap the next window's QK stream), or inline
                    # for the final window
                    if drain:
                        for j in range(NT - LAG, NT):
                            pv(j)
                        # this window's norm reuses "on" slots of window
                        # w-2; their transpose readers must emit first
                        for nm in pre_norm_needs:
                            need(nm)
                        recips()
                        return onA, onB, norm_t
                    for i, j in enumerate(range(NT - LAG, NT)):
                        add_thunk(f"pv{w}_{j}", lambda j=j: pv(j),
                                  250.0, (w + 1, 0.1 + 0.35 * i))
                    add_thunk(f"norm{w}", norm, 50.0, (w + 1, 3.0))
                    return onA, onB, None

                # ---- transpose O -> O^T thunks ----------------------------
                def make_transposes(mm, ig, onA, onB, tail=False):
                    ots = [None] * NI

                    def tr(t):
                        T = pb.tile([128, 512], f32, name="T",
                                    tag=("s_ps" if tail else "op"), bufs=2)
                        Tf = T.bitcast(fp16)
                        nc.tensor.transpose(Tf[0:64, 0:128], onA[:, t, :],
                                            ident[:])
                        nc.tensor.transpose(Tf[64:128, 0:128], onB[:, t, :],
                                            ident[:])
                        ot = op.tile([128, 128], fp16, name="ot", tag="ot",
                                     bufs=32)
                        nc.vector.tensor_copy(out=ot[:], in_=Tf[:, 0:128])
                        ots[t] = ot

                    thunks = [(f"tr{mm}_{ig}_{t}", lambda t=t: tr(t), 120.0)
                              for t in range(NI)]
                    return thunks, ots

                # ---- output projection ------------------------------------
                def make_outproj(ig, ots0, ots1, tail=False):
                    out_sb = op.tile([128, 4096], fp16, name="out_sb",
                                     tag="out_sb", bufs=2)

                    def group(gi):
                        jt, dch = gi // 2, gi % 2
                        dsl = slice(dch * 512, (dch + 1) * 512)
                        ptag = "s_ps" if (tail and gi % 2 == 0) else "op"
                        out_ps = pb.tile([128, 512], f32, name="out_ps",
                                         tag=ptag, bufs=2)
                        nc.tensor.matmul(out_ps[:, 0:512], ots0[jt][:],
                                         wo_sb[0][:, dsl],
                                         start=True, stop=False)
                        nc.tensor.matmul(out_ps[:, 0:512], ots1[jt][:],
                                         wo_sb[1][:, dsl],
                                         start=False, stop=True)
                        csl = slice(jt * 1024 + dch * 512,
                                    jt * 1024 + (dch + 1) * 512)
                        if tail and gi % 2 == 0:
                            nc.scalar.copy(out=out_sb[:, csl],
                                           in_=out_ps[:, 0:512])
                        else:
                            nc.vector.tensor_copy(out=out_sb[:, csl],
                                                  in_=out_ps[:, 0:512])

                    def ship(jt0, jt1):
                        part_v = part_d[ig * 512 + jt0 * 128:
                                        ig * 512 + jt1 * 128,
                                        :].rearrange(
                            "(jt p) d -> p jt d", p=128)
                        nc.sync.dma_start(
                            out=part_v,
                            in_=out_sb[:, jt0 * 1024:jt1 * 1024]
                            .rearrange("p (jt d) -> p jt d", jt=jt1 - jt0))

                    def ship_t(jt):
                        rows = slice(ig * 512 + jt * 128,
                                     ig * 512 + (jt + 1) * 128)
                        nc.sync.dma_start(
                            out=part_d[rows, :],
                            in_=out_sb[:, jt * 1024:(jt + 1) * 1024])

                    return group, ship, ship_t

                # ---- main flow --------------------------------------------
                windows = [(0, i) for i in range(NI)] + \
                          [(1, i) for i in range(NI)]
                ots0 = {}
                for w, (m, ig) in enumerate(windows):
                    last = (w == len(windows) - 1)
                    pre = ([f"tr{windows[w - 2][0]}_{windows[w - 2][1]}_{t}"
                            for t in range(NI)] if last else ())
                    onA, onB, norm_t = attention(m, ig, w, drain=last,
                                                 pre_norm_needs=pre)
                    thunks, ots = make_transposes(m, ig, onA, onB,
                                                  tail=last)
                    if m == 0:
                        for t, (nm, fn, cost) in enumerate(thunks):
                            add_thunk(nm, fn, cost, (w + 1, 6 + t))
                        ots0[ig] = ots
                        continue
                    if not last:
                        for t, (nm, fn, cost) in enumerate(thunks):
                            add_thunk(nm, fn, cost, (w + 1, 3.5 + 0.8 * t))
                        group, ship, _ = make_outproj(ig, ots0[ig], ots)
                        for gi in range(8):
                            if ig < NI - 2:
                                dl = ((w + 1, 6.5 + 0.9 * gi) if gi < 6
                                      else (w + 2, 1 + 0.8 * (gi - 6)))
                            else:
                                # the last deferred chunk must fully drain
                                # inside the final window
                                dl = (w + 1, 6.5 + 0.85 * gi)
                            add_thunk(f"opj{ig}_{gi}",
                                      lambda g=gi, grp=group: grp(g), 430.0,
                                      dl)
                        add_thunk(f"ship{ig}", lambda s=ship: s(0, 4), 0.0,
                                  (w + 2, 2.7) if ig < NI - 2
                                  else (w + 1, 13.8))
                    else:
                        while heap:
                            pop_one()
                        group, ship, _ = make_outproj(
                            ig, ots0[ig], ots, tail=True)
                        for t in range(NI):
                            norm_t(t)
                            thunks[t][1]()
                            group(2 * t)
                            group(2 * t + 1)
                            if t == 1:
                                ship(0, 2)
                            elif t >= 2:
                                ship(t, t + 1)


def _get_nc():
    if "nc" not in _CACHE:
        _CACHE["nc"] = build_program()
    return _CACHE["nc"]


def make_in_maps(x, Wq, Wkv, Wo):
    import ml_dtypes

    bf16 = ml_dtypes.bfloat16
    scale = DH ** -0.5
    x = np.asarray(x, dtype=np.float32)
    Wq = np.asarray(Wq, dtype=np.float32)
    Wkv = np.asarray(Wkv, dtype=np.float32)
    Wo = np.asarray(Wo, dtype=np.float32)
    xt = [np.ascontiguousarray(x[b].T.astype(bf16)) for b in range(B)]
    in_maps = []
    for c in range(NCORES):
        b, hg = c // HPG, c % HPG
        sl = slice(hg * DGRP, (hg + 1) * DGRP)
        in_maps.append({
            "xt": xt[b],
            "wq": np.ascontiguousarray((Wq[:, sl] * scale).astype(bf16)),
            "wk": np.ascontiguousarray(
                Wkv[:, hg * DGRP:(hg + 1) * DGRP].astype(bf16)),
            "wv": np.ascontiguousarray(
                Wkv[:, DIM + hg * DGRP:DIM + (hg + 1) * DGRP].astype(bf16)),
            "wo": np.ascontiguousarray(Wo[sl, :].astype(np.float16)),
        })
    return in_maps


def combine_outputs(results, bo):
    out = np.zeros((B, N, DIM), dtype=np.float32)
    for c in range(NCORES):
        out[c // HPG] += results[c]["part"]
    out += np.asarray(bo, dtype=np.float32)
    return out


def kernel(x, mask, Wq, Wkv, Wo, bo):
    from concourse.bass_utils import run_bass_kernel_spmd

    nc = _get_nc()
    in_maps = make_in_maps(x, Wq, Wkv, Wo)
    res = run_bass_kernel_spmd(nc, in_maps, list(range(NCORES)))
    return combine_outputs(res.results, bo)


# revision 86
# speedup vs baseline: 1.2128x; 1.0015x over previous
"""Trainium2 Bass kernel for nn_Attention_24008867185039.

Reference computation (B=2, N=2048, DIM=1024, 16 heads x 64):
    q = x @ Wq ; k, v = split(x @ Wkv) ; per-head softmax(q k^T / sqrt(64)) v
    out = attn_out @ Wo + bo
(mask is all-ones per the problem spec, so masking is a no-op.)

Sharding (8 cores): data-parallel over batch (2) x tensor-parallel over 4
head-groups of 4 heads. Host sums the 4 Wo partials per batch, adds bias.

Cost-model shape (per core): the scalar engine's exp stream (128 ops of
[128,1024], ~137us) and the PE stream (~142us) are co-critical.  PE work is
minimized by exploiting that matmul cost is charged per MOVING row only
(stationary loads are free):
  - QK^T: stationary K-tile [64d, 128keys], moving Q [64d, 512q] -> S^T
    [128 keys, 512 q] per (head, j).  (d=64 makes 50% PE util unavoidable.)
  - PV: stationary exp-tile [128 keys, 128 q], moving V [128 keys, 65]
    (64 dims + a ones column that yields the softmax row-sum for free) ->
    O [128 q, 65] accumulated over the 16 key tiles.  65 moving rows per
    matmul instead of 512 halves the attention-PV cost vs the naive layout.
  - O lands as [q, d']: normalization is a native per-partition
    reciprocal + tensor_scalar multiply, then a PE transpose (128 rows)
    rebuilds O^T [d', q] for the (full-util) output projection.
Projections and the output projection are at the full-utilization floor.

Precision: x/Wq/Wkv load as bf16; S^T in f32 PSUM; exp/V/O^T/Wo in fp16
(same PE rate as bf16, 8x finer mantissa).

Schedule: dummy matmuls ramp the PE p-state while the first x/weight DMAs
land (all DMA transfers serialize on one engine pool in the cost model);
a minimal prefix (K^T cols 0:128 of pair0 + Q^T(pair0,chunk0)) starts the
exp stream ~12us in.  Every other projection chain, V tile, transpose and
output-projection group is a deadline-sorted thunk dripped through the
attention windows' PE slack under a credit scheduler (~440ns of drip
budget per exp), with explicit need() guards enforcing producer-before-
consumer emission order.  PV runs LAG=8 key-tiles behind QK; each
window's PV tail + normalization are deferred into the next window so the
exp stream never waits at window boundaries.  PSUM accumulation groups
are bank-granular (start marks the whole 2KB zero region), so each O bank
carries one group per window, and the next window's bank-claiming start
takes an explicit dep on the previous normalization's last DVE read.
PSUM: s 2x[128,1024] + O-accum 2x[128,4x65] + a shared 2-bank ring for
proj chains / transposes / outproj tiles = 8 banks.
"""

import sys

sys.path.insert(0, "/opt/trn_rl_repo")

import numpy as np

B, N, DIM, HEADS, DH = 2, 2048, 1024, 16, 64
HPG = 4                 # heads per core (head group)
DGRP = HPG * DH         # 256: per-core slice of the inner dim
NCORES = 8
KT = DIM // 128         # 8 contraction tiles for projections
NT = N // 128           # 16 sequence tiles of 128
NI = N // 512           # 4 query chunks of 512
MT = DGRP // 128        # 2 head-pair tiles per core
LAG = 8                 # PV trails QK by this many key tiles
WARMUP = 36             # PE p-state ramp matmuls during the first DMAs
SLACK_NS = 440          # drip budget granted per exp op

_CACHE = {}


def build_program(repeats=1):
    import concourse.mybir as mybir
    import concourse.tile as tile
    from concourse import bacc

    f32 = mybir.dt.float32
    bf16 = mybir.dt.bfloat16
    fp16 = mybir.dt.float16

    nc = bacc.Bacc("TRN2", target_bir_lowering=False, debug=False,
                   num_devices=NCORES)

    xt_d = nc.dram_tensor("xt", [DIM, N], bf16, kind="ExternalInput").ap()
    wq_d = nc.dram_tensor("wq", [DIM, DGRP], bf16, kind="ExternalInput").ap()
    wk_d = nc.dram_tensor("wk", [DIM, DGRP], bf16, kind="ExternalInput").ap()
    wv_d = nc.dram_tensor("wv", [DIM, DGRP], bf16, kind="ExternalInput").ap()
    wo_d = nc.dram_tensor("wo", [DGRP, DIM], fp16, kind="ExternalInput").ap()
    part_d = nc.dram_tensor("part", [N, DIM], fp16,
                            kind="ExternalOutput").ap()

    with tile.TileContext(nc) as tc:
        for rep in range(repeats):
            _emit_body(nc, tc, xt_d, wq_d, wk_d, wv_d, wo_d, part_d,
                       tag=f"r{rep}")

    nc.compile()
    return nc


def _emit_body(nc, tc, xt_d, wq_d, wk_d, wv_d, wo_d, part_d, tag):
    import concourse.mybir as mybir
    from concourse.masks import make_identity
    from concourse.tile_rust import add_dep_helper

    f32 = mybir.dt.float32
    bf16 = mybir.dt.bfloat16
    fp16 = mybir.dt.float16
    Exp = mybir.ActivationFunctionType.Exp

    xt_t = xt_d.rearrange("(t p) n -> p t n", p=128)    # [128, KT, N]
    wq_t = wq_d.rearrange("(t p) d -> p t d", p=128)    # [128, KT, DGRP]
    wk_t = wk_d.rearrange("(t p) d -> p t d", p=128)
    wv_t = wv_d.rearrange("(t p) d -> p t d", p=128)
    wo_t = wo_d.rearrange("(t p) d -> t p d", p=128)    # [MT, 128, DIM]

    with nc.allow_low_precision(reason="fp16/bf16 rounding is intentional"):
        with tc.tile_pool(name=f"pp{tag}", bufs=1) as pp, \
             tc.tile_pool(name=f"sp{tag}", bufs=4) as sp, \
             tc.tile_pool(name=f"ep{tag}", bufs=16) as ep, \
             tc.tile_pool(name=f"osb{tag}", bufs=2) as op:

            # ---- persistent SBUF ------------------------------------------
            # x^T lives as one tile per 512-token chunk so consumers only
            # depend on the DMA that actually feeds them
            xt_sb = [pp.tile([128, KT, 512], bf16, name=f"xt_sb{c}")
                     for c in range(NI)]
            wq_sb = pp.tile([128, KT, DGRP], bf16, name="wq_sb")
            wk_sb = pp.tile([128, KT, DGRP], bf16, name="wk_sb")
            wv_sb = pp.tile([128, KT, DGRP], bf16, name="wv_sb")
            wo_sb = [pp.tile([128, DIM], fp16, name=f"wo_sb{m}")
                     for m in range(MT)]
            qt = [[pp.tile([128, 512], fp16, name=f"qt{m}_{i}")
                   for i in range(NI)] for m in range(MT)]
            kt = [pp.tile([128, N], fp16, name=f"kt{m}") for m in range(MT)]
            # V with a ones column per head: [128, 4 heads x (64 d + 1)]
            vsb = [pp.tile([128, HPG * (DH + 1)], fp16, name=f"vsb{j}")
                   for j in range(NT)]
            ident = pp.tile([128, 128], fp16, name="ident")
            # warmup operand, independent of ident so the PE ramp matmuls
            # don't wait on the Pool engine's startup memset queue
            wrm = pp.tile([128, 128], fp16, name="wrm")
            nc.vector.memset(wrm[:], 0.125)
            # touch Exp once so the activation table is resident before the
            # first real exp (the lazy load would otherwise delay it); uses
            # its own tiny tile so the warmup matmuls don't wait on it
            pre = pp.tile([1, 2], fp16, name="pre")
            nc.vector.memset(pre[:], 0.1)
            nc.scalar.activation(out=pre[0:1, 0:1], in_=pre[0:1, 1:2],
                                 func=mybir.ActivationFunctionType.Exp)
            make_identity(nc, ident[:])

            # ---- DMAs (ordered by first need) -----------------------------
            # All transfers serialize on the DMA-engine pool, and runs
            # under 512B get half bandwidth, so weights go as full-width
            # transfers and x's first chunk in two 256-col (512B-run) pieces
            nc.sync.dma_start(out=wk_sb[:], in_=wk_t)
            nc.sync.dma_start(out=xt_sb[0][:, :, 0:256],
                              in_=xt_t[:, :, 0:256])
            nc.sync.dma_start(out=wq_sb[:], in_=wq_t)
            nc.sync.dma_start(out=xt_sb[0][:, :, 256:512],
                              in_=xt_t[:, :, 256:512])
            nc.sync.dma_start(out=wv_sb[:], in_=wv_t)
            nc.sync.dma_start(out=xt_sb[1][:], in_=xt_t[:, :, 512:1024])
            nc.sync.dma_start(out=xt_sb[2][:], in_=xt_t[:, :, 1024:1536])
            nc.sync.dma_start(out=xt_sb[3][:], in_=xt_t[:, :, 1536:2048])
            for m in range(MT):
                nc.sync.dma_start(out=wo_sb[m][:], in_=wo_t[m])

            # ---- thunk scheduler (deadline-ordered drip queue) ------------
            import heapq

            def v3(j):
                return vsb[j].rearrange("p (h c) -> p h c", h=HPG)

            emitted = set()
            heap = []
            state = {"credit": 0.0, "seq": 0}

            def add_thunk(name, fn, cost, deadline):
                state["seq"] += 1
                heapq.heappush(heap, (deadline, state["seq"], name, fn,
                                      cost))

            def pop_one():
                _, _, name, fn, cost = heapq.heappop(heap)
                fn()
                emitted.add(name)
                # debt floor: a forced overdraw stalls the exp stream once;
                # later windows shouldn't keep paying for it
                state["credit"] = max(state["credit"] - cost, -1200.0)

            def drip(budget_pops=3, force=False):
                # cap stops surplus from bursting several projection chains
                # into one slot (which would starve the exp stream)
                state["credit"] = min(state["credit"] + SLACK_NS, 1300.0)
                n = 0
                while heap and n < budget_pops and (
                        force or state["credit"] >= heap[0][4]):
                    pop_one()
                    n += 1

            def need(name):
                while name not in emitted:
                    assert heap, f"thunk {name} was never queued"
                    pop_one()

            # ---- Phase A: warmup + minimal prefix -------------------------
            with tc.tile_pool(name=f"pa{tag}", bufs=1, space="PSUM") as pa:
                scratch = pa.tile([128, 128], f32, name="scratch")
                for _ in range(WARMUP):
                    nc.tensor.matmul(scratch[:], wrm[:], wrm[:],
                                     start=True, stop=True)
                # K^T(pair0) cols 0:128 only — just enough for QK(j=0)
                kc_ps = pa.tile([128, 128], f32, name="kc_ps")
                for k in range(KT):
                    nc.tensor.matmul(kc_ps[:], wk_sb[:, k, 0:128],
                                     xt_sb[0][:, k, 0:128],
                                     start=(k == 0), stop=(k == KT - 1))
                nc.vector.tensor_copy(out=kt[0][:, 0:128], in_=kc_ps[:])
                emitted.add("kt0p0")
                # keys 128:256 need only xt0a too — fill the wq DMA wait
                kp_ps = pa.tile([128, 128], f32, name="kp_ps")
                for k in range(KT):
                    nc.tensor.matmul(kp_ps[:], wk_sb[:, k, 0:128],
                                     xt_sb[0][:, k, 128:256],
                                     start=(k == 0), stop=(k == KT - 1))
                nc.vector.tensor_copy(out=kt[0][:, 128:256], in_=kp_ps[:])
                emitted.add("kt0p1")
                for _ in range(8):
                    nc.tensor.matmul(scratch[:], wrm[:], wrm[:],
                                     start=True, stop=True)
                # Q^T(pair0) chunk0, in two half-chains pipelined against
                # the two x column-piece DMAs
                for h, csl in enumerate((slice(0, 256), slice(256, 512))):
                    q_ps = pa.tile([128, 256], f32, name=f"q_ps{h}")
                    for k in range(KT):
                        nc.tensor.matmul(q_ps[:], wq_sb[:, k, 0:128],
                                         xt_sb[0][:, k, csl],
                                         start=(k == 0), stop=(k == KT - 1))
                    nc.vector.tensor_copy(out=qt[0][0][:, csl], in_=q_ps[:])
                    if h == 0:
                        for _ in range(19):
                            nc.tensor.matmul(scratch[:], wrm[:], wrm[:],
                                             start=True, stop=True)
                emitted.add("qt00")

            # ---- dripped projection thunks --------------------------------
            with tc.tile_pool(name=f"pb{tag}", bufs=2, space="PSUM") as pb:

                def proj_cols(w_sb, msl, ich, csl, dst_copy):
                    p_ps = pb.tile([128, 512], f32, name="p_ps", tag="op",
                                   bufs=2)
                    ncols = csl.stop - csl.start
                    for k in range(KT):
                        nc.tensor.matmul(p_ps[:, 0:ncols],
                                         w_sb[:, k, msl],
                                         xt_sb[ich][:, k, csl],
                                         start=(k == 0), stop=(k == KT - 1))
                    dst_copy(p_ps[:, 0:ncols])

                def kt_cols(m, ich, c0, c1):
                    gsl = slice(ich * 512 + c0, ich * 512 + c1)
                    proj_cols(
                        wk_sb, slice(m * 128, (m + 1) * 128), ich,
                        slice(c0, c1),
                        lambda p: nc.vector.tensor_copy(out=kt[m][:, gsl],
                                                        in_=p))

                def qt_cols(m, ich, c0, c1):
                    proj_cols(
                        wq_sb, slice(m * 128, (m + 1) * 128), ich,
                        slice(c0, c1),
                        lambda p: nc.vector.tensor_copy(
                            out=qt[m][ich][:, c0:c1], in_=p))

                def v_tile(pair, j):
                    csl = slice((j % 4) * 128, (j % 4) * 128 + 128)
                    msl = slice(pair * 128, (pair + 1) * 128)
                    v_ps = pb.tile([128, 512], f32, name="vp_ps", tag="op",
                                   bufs=2)
                    for k in range(KT):
                        nc.tensor.matmul(v_ps[:, 0:128],
                                         xt_sb[j // 4][:, k, csl],
                                         wv_sb[:, k, msl],
                                         start=(k == 0), stop=(k == KT - 1))
                    nc.vector.tensor_copy(
                        out=v3(j)[:, 2 * pair:2 * pair + 2, 0:DH],
                        in_=v_ps[:, 0:128].rearrange("p (h c) -> p h c", h=2))
                    nc.gpsimd.memset(
                        v3(j)[:, 2 * pair:2 * pair + 2, DH:DH + 1], 1.0)

                QRT, VCOST = 427.0, 427.0

                def add_qt(m, ich, name, dls):
                    # quarter chains; the full-chunk name goes on the last
                    # quarter so need(name) forces all four
                    for q in range(4):
                        add_thunk(name + ("" if q == 3 else f"_{q}"),
                                  lambda q=q: qt_cols(m, ich, q * 128,
                                                      (q + 1) * 128),
                                  QRT, dls[q])

                # prefix extension: kt0 key-pieces 1-3 run in the PE gaps
                # while the first DMAs land (emitted by the code below, not
                # dripped)
                def kt0_piece(j):
                    kt_cols(0, j // 4, (j % 4) * 128, (j % 4) * 128 + 128)

                for j in (2, 3):
                    add_thunk(f"kt0p{j}", lambda j=j: kt0_piece(j), VCOST,
                              (0, j - 1.8))
                # pair-0 K^T in 128-col pieces so the forced drip in the
                # first window stays fine-grained
                for j in range(4, NT):
                    add_thunk(f"kt0p{j}", lambda j=j: kt0_piece(j),
                              VCOST, (0, j - 0.5))
                # V tiles whose PV consumer is deferred into the next window
                # spill their deadline there too, interleaved with the
                # deferred PV thunks that consume them
                for j in range(NT):
                    # spilled V tiles must sort BEFORE their deferred-PV
                    # consumer, which must sort before that window's norm
                    dl = (0, j + 7.7) if j <= 7 else (1, 0.35 * (j - 8))
                    add_thunk(f"v0_{j}", lambda j=j: v_tile(0, j), VCOST,
                              dl)
                add_qt(0, 1, "qt01", [(0, 8), (0, 9.7), (0, 11.4),
                                      (0, 13.1)])
                for h in range(2 * NI):
                    w_, o_ = (1, 6) if h < 2 else ((2, 6) if h < 4
                                                   else (3, 1))
                    add_thunk(f"kt1c{h // 2}" + ("" if h % 2 else "a"),
                              lambda h=h: kt_cols(1, h // 2, (h % 2) * 256,
                                                  (h % 2) * 256 + 256),
                              853.0, (w_, o_ + 3 * (h % 2 if h < 4
                                                    else h - 4)))
                add_qt(0, 2, "qt02", [(1, 7), (1, 9), (1, 11), (1, 13)])
                for j in range(0, 8):
                    add_thunk(f"v1_{j}", lambda j=j: v_tile(1, j), VCOST,
                              (2, 4 + 0.9 * j))
                add_qt(0, 3, "qt03", [(2, 7), (2, 9), (2, 11), (2, 13)])
                for j in range(8, NT):
                    add_thunk(f"v1_{j}", lambda j=j: v_tile(1, j), VCOST,
                              (5, 0.35 * (j - 8)))
                add_qt(1, 0, "qt10", [(3, 5), (3, 7.5), (3, 10), (3, 12.5)])
                add_qt(1, 1, "qt11", [(4, 5), (4, 7), (4, 9), (4, 11)])
                add_qt(1, 2, "qt12", [(5, 5), (5, 7), (5, 9), (5, 11)])
                add_qt(1, 3, "qt13", [(6, 5), (6, 7), (6, 9), (6, 11)])

                # ---- attention window -------------------------------------
                def attention(m, ig, w, drain=False, pre_norm_needs=()):
                    need(f"qt{m}{ig}" if (m, ig) != (0, 0) else "qt00")
                    vA = slice(2 * m * (DH + 1), 2 * m * (DH + 1) + DH + 1)
                    vB = slice((2 * m + 1) * (DH + 1),
                               (2 * m + 1) * (DH + 1) + DH + 1)
                    oA = pb.tile([128, NI, DH + 1], f32, name="oA", tag="oA",
                                 bufs=1)
                    oB = pb.tile([128, NI, DH + 1], f32, name="oB", tag="oB",
                                 bufs=1)
                    onA = op.tile([128, NI, DH], fp16, name="onA", tag="on",
                                  bufs=4)
                    onB = op.tile([128, NI, DH], fp16, name="onB", tag="on",
                                  bufs=4)
                    es = [None] * NT

                    def qk(j):
                        if m == 0:
                            need(f"kt0p{j}")
                        else:
                            need(f"kt{m}c{j // 4}")
                        jsl = slice(j * 128, (j + 1) * 128)
                        s_ps = pb.tile([128, 1024], f32, name="s_ps",
                                       tag="s_ps", bufs=2)
                        nc.tensor.matmul(s_ps[:, 0:512], kt[m][0:64, jsl],
                                         qt[m][ig][0:64, :],
                                         start=True, stop=True)
                        nc.tensor.matmul(s_ps[:, 512:1024],
                                         kt[m][64:128, jsl],
                                         qt[m][ig][64:128, :],
                                         start=True, stop=True)
                        e = ep.tile([128, 1024], fp16, name="e_sb")
                        nc.scalar.activation(out=e[:], in_=s_ps[:], func=Exp)
                        es[j] = e

                    def pv(j):
                        need(f"v{m}_{j}")
                        e = es[j]
                        # one accumulation group per O bank: start marks the
                        # whole 2KB zero region, so only the first matmul of
                        # the window starts and only the last stops
                        for t in range(NI):
                            mm = nc.tensor.matmul(
                                oA[:, t, :], e[:, t * 128:(t + 1) * 128],
                                vsb[j][:, vA],
                                start=(j == 0 and t == 0),
                                stop=(j == NT - 1 and t == NI - 1))
                            if j == 0 and t == 0 and state.get("norm_last"):
                                # the bank-claiming start must wait for the
                                # previous window's normalization reads (the
                                # region-based tracker only sees subtile 0)
                                add_dep_helper(
                                    mm.ins, state["norm_last"].ins,
                                    reason="O-bank WAR vs prev norm")
                        for t in range(NI):
                            nc.tensor.matmul(
                                oB[:, t, :],
                                e[:, 512 + t * 128:512 + (t + 1) * 128],
                                vsb[j][:, vB],
                                start=(j == 0 and t == 0),
                                stop=(j == NT - 1 and t == NI - 1))

                    rr_box = {}

                    def recips():
                        # O[:, :, 64] holds the softmax row-sums
                        rr = sp.tile([128, 8], f32, name="rr", tag="rr")
                        nc.vector.reciprocal(rr[:, 0:4], oA[:, :, DH])
                        nc.vector.reciprocal(rr[:, 4:8], oB[:, :, DH])
                        rr_box["rr"] = rr

                    def norm_t(t):
                        rr = rr_box["rr"]
                        nc.vector.tensor_scalar_mul(
                            onA[:, t, :], oA[:, t, 0:DH], rr[:, t:t + 1])
                        state["norm_last"] = nc.vector.tensor_scalar_mul(
                            onB[:, t, :], oB[:, t, 0:DH], rr[:, 4 + t:5 + t])

                    def norm():
                        recips()
                        # t descending: subtile 0 is read LAST on the
                        # in-order DVE, and the next window's first PV
                        # matmul (whose bank-claiming start only region-
                        # depends on subtile 0) then transitively waits for
                        # all of this window's normalization reads
                        for t in reversed(range(NI)):
                            norm_t(t)

                    for j in range(NT):
                        if j == LAG and w >= 1:
                            # previous window's deferred PV tail + norm must
                            # emit before this window reuses the O banks or
                            # their exp tiles' SBUF slots
                            need(f"norm{w - 1}")
                        qk(j)
                        if j < NT - 1:
                            drip(budget_pops=(2 if drain and j >= 13
                                              else 3),
                                 force=(drain and j >= 13))
                        if j >= LAG:
                            pv(j - LAG)

                    # tail of the PV stream + normalization either run as
                    # thunks early in the next window (so this window's last
                    # exps overlap the next window's QK stream), or inline
                    # for the final window
                    if drain:
                        for j in range(NT - LAG, NT):
                            pv(j)
                        # this window's norm reuses "on" slots of window
                        # w-2; their transpose readers must emit first
                        for nm in pre_norm_needs:
                            need(nm)
                        recips()
                        return onA, onB, norm_t
                    for i, j in enumerate(range(NT - LAG, NT)):
                        add_thunk(f"pv{w}_{j}", lambda j=j: pv(j),
                                  250.0, (w + 1, 0.1 + 0.35 * i))
                    add_thunk(f"norm{w}", norm, 50.0, (w + 1, 3.0))
                    return onA, onB, None

                # ---- transpose O -> O^T thunks ----------------------------
                def make_transposes(mm, ig, onA, onB, tail=False):
                    ots = [None] * NI

                    def tr(t):
                        T = pb.tile([128, 512], f32, name="T",
                                    tag=("s_ps" if tail else "op"), bufs=2)
                        Tf = T.bitcast(fp16)
                        nc.tensor.transpose(Tf[0:64, 0:128], onA[:, t, :],
                                            ident[:])
                        nc.tensor.transpose(Tf[64:128, 0:128], onB[:, t, :],
                                            ident[:])
                        ot = op.tile([128, 128], fp16, name="ot", tag="ot",
                                     bufs=32)
                        nc.vector.tensor_copy(out=ot[:], in_=Tf[:, 0:128])
                        ots[t] = ot

                    thunks = [(f"tr{mm}_{ig}_{t}", lambda t=t: tr(t), 120.0)
                              for t in range(NI)]
                    return thunks, ots

                # ---- output projection ------------------------------------
                def make_outproj(ig, ots0, ots1, tail=False):
                    out_sb = op.tile([128, 4096], fp16, name="out_sb",
                                     tag="out_sb", bufs=2)

                    def group(gi):
                        jt, dch = gi // 2, gi % 2
                        dsl = slice(dch * 512, (dch + 1) * 512)
                        ptag = "s_ps" if (tail and gi % 2 == 0) else "op"
                        out_ps = pb.tile([128, 512], f32, name="out_ps",
                                         tag=ptag, bufs=2)
                        nc.tensor.matmul(out_ps[:, 0:512], ots0[jt][:],
                                         wo_sb[0][:, dsl],
                                         start=True, stop=False)
                        nc.tensor.matmul(out_ps[:, 0:512], ots1[jt][:],
                                         wo_sb[1][:, dsl],
                                         start=False, stop=True)
                        csl = slice(jt * 1024 + dch * 512,
                                    jt * 1024 + (dch + 1) * 512)
                        if tail and gi % 2 == 0:
                            nc.scalar.copy(out=out_sb[:, csl],
                                           in_=out_ps[:, 0:512])
                        else:
                            nc.vector.tensor_copy(out=out_sb[:, csl],
                                                  in_=out_ps[:, 0:512])

                    def ship(jt0, jt1):
                        part_v = part_d[ig * 512 + jt0 * 128:
                                        ig * 512 + jt1 * 128,
                                        :].rearrange(
                            "(jt p) d -> p jt d", p=128)
                        nc.sync.dma_start(
                            out=part_v,
                            in_=out_sb[:, jt0 * 1024:jt1 * 1024]
                            .rearrange("p (jt d) -> p jt d", jt=jt1 - jt0))

                    def ship_t(jt):
                        rows = slice(ig * 512 + jt * 128,
                                     ig * 512 + (jt + 1) * 128)
                        nc.sync.dma_start(
                            out=part_d[rows, :],
                            in_=out_sb[:, jt * 1024:(jt + 1) * 1024])

                    return group, ship, ship_t

                # ---- main flow --------------------------------------------
                windows = [(0, i) for i in range(NI)] + \
                          [(1, i) for i in range(NI)]
                ots0 = {}
                for w, (m, ig) in enumerate(windows):
                    last = (w == len(windows) - 1)
                    pre = ([f"tr{windows[w - 2][0]}_{windows[w - 2][1]}_{t}"
                            for t in range(NI)] if last else ())
                    onA, onB, norm_t = attention(m, ig, w, drain=last,
                                                 pre_norm_needs=pre)
                    thunks, ots = make_transposes(m, ig, onA, onB,
                                                  tail=last)
                    if m == 0:
                        # norm muls finish t-descending, so transposes pop
                        # t-descending too to avoid head-of-line DVE waits
                        for t, (nm, fn, cost) in enumerate(thunks):
                            add_thunk(nm, fn, cost, (w + 1, 6 + (3 - t)))
                        ots0[ig] = ots
                        continue
                    if not last:
                        for t, (nm, fn, cost) in enumerate(thunks):
                            add_thunk(nm, fn, cost,
                                      (w + 1, 3.5 + 0.8 * (3 - t)))
                        group, ship, _ = make_outproj(ig, ots0[ig], ots)
                        for gi in range(8):
                            if ig < NI - 2:
                                dl = ((w + 1, 6.5 + 0.9 * gi) if gi < 6
                                      else (w + 2, 1 + 0.8 * (gi - 6)))
                            else:
                                # the last deferred chunk must fully drain
                                # inside the final window
                                dl = (w + 1, 6.5 + 0.85 * gi)
                            add_thunk(f"opj{ig}_{gi}",
                                      lambda g=gi, grp=group: grp(g), 430.0,
                                      dl)
                        add_thunk(f"ship{ig}", lambda s=ship: s(0, 4), 0.0,
                                  (w + 2, 2.7) if ig < NI - 2
                                  else (w + 1, 13.8))
                    else:
                        while heap:
                            pop_one()
                        group, ship, _ = make_outproj(
                            ig, ots0[ig], ots, tail=True)
                        for t in range(NI):
                            norm_t(t)
                            thunks[t][1]()
                            group(2 * t)
                            group(2 * t + 1)
                            if t == 1:
                                ship(0, 2)
                            elif t >= 2:
                                ship(t, t + 1)


def _get_nc():
    if "nc" not in _CACHE:
        _CACHE["nc"] = build_program()
    return _CACHE["nc"]


def make_in_maps(x, Wq, Wkv, Wo):
    import ml_dtypes

    bf16 = ml_dtypes.bfloat16
    scale = DH ** -0.5
    x = np.asarray(x, dtype=np.float32)
    Wq = np.asarray(Wq, dtype=np.float32)
    Wkv = np.asarray(Wkv, dtype=np.float32)
    Wo = np.asarray(Wo, dtype=np.float32)
    xt = [np.ascontiguousarray(x[b].T.astype(bf16)) for b in range(B)]
    in_maps = []
    for c in range(NCORES):
        b, hg = c // HPG, c % HPG
        sl = slice(hg * DGRP, (hg + 1) * DGRP)
        in_maps.append({
            "xt": xt[b],
            "wq": np.ascontiguousarray((Wq[:, sl] * scale).astype(bf16)),
            "wk": np.ascontiguousarray(
                Wkv[:, hg * DGRP:(hg + 1) * DGRP].astype(bf16)),
            "wv": np.ascontiguousarray(
                Wkv[:, DIM + hg * DGRP:DIM + (hg + 1) * DGRP].astype(bf16)),
            "wo": np.ascontiguousarray(Wo[sl, :].astype(np.float16)),
        })
    return in_maps


def combine_outputs(results, bo):
    out = np.zeros((B, N, DIM), dtype=np.float32)
    for c in range(NCORES):
        out[c // HPG] += results[c]["part"]
    out += np.asarray(bo, dtype=np.float32)
    return out


def kernel(x, mask, Wq, Wkv, Wo, bo):
    from concourse.bass_utils import run_bass_kernel_spmd

    nc = _get_nc()
    in_maps = make_in_maps(x, Wq, Wkv, Wo)
    res = run_bass_kernel_spmd(nc, in_maps, list(range(NCORES)))
    return combine_outputs(res.results, bo)
